# revision 1
# baseline (speedup 1.0000x reference)
"""Qwen-style GQA full attention (B=2, S=2048, HID=2048, H=16, KVH=8, D=128)
on 8 trn2 NeuronCores.

Sharding: tensor-parallel across head groups. Core d owns kv-head d and its
two query heads (2d, 2d+1): Wq/Wk/Wv column shards, Wo row shard. Each core
computes a partial [B*S, HID] output (its 2 heads' contribution through its
Wo row block); the host sums the 8 partials.

Device kernel (per core, all matmuls bf16, fp32 PSUM accumulation):
  phase 1  QKV+gate projection, feature-major ([feat, tok]) via stationary
           W-chunks against moving hsT (host-pretransposed hidden states).
           Per-head RMSNorm done with a ones-vector partition-sum matmul +
           exp(-0.5*ln(ss/128+eps)); RoPE via half-rotated sin/cos tables
           (norm weight + 1/sqrt(D) folded in host-side). Gate sigmoid is
           computed as exp(-ln(1+exp(-g))) so the scalar engine only ever
           needs the natural_log_exp table set.
  phase 2  V transposed to token-major via PE transposes.
  phase 3  causal attention per (batch, q-tile-pair): scoresT = K-chunk
           stationary x moving Q -> exp -> diagonal-block masking (exact
           zeros) -> PV and broadcast row-sum accumulation; out columns are
           rescaled by exp(-ln(sum)) (softmax denominator; no max
           subtraction needed since RMS-normed q,k bound |score|<=sqrt(D)).
  phase 4  sigmoid-gate multiply + Wo row-shard projection -> partial out.
"""

import os
import numpy as np
import ml_dtypes

import concourse.bass as bass
import concourse.tile as tile
from concourse import bacc, mybir
from contextlib import ExitStack

BF16 = ml_dtypes.bfloat16
F32 = mybir.dt.float32
BF = mybir.dt.bfloat16
AF = mybir.ActivationFunctionType

class _Bacc(bacc.Bacc):
    """Bacc that prefers the combined Ln+Exp activation table set, so the
    kernel's Ln/Exp/Copy mix resolves to a single ACT_TABLE_LOAD instead of
    thrashing between exp_and_others and natural_log (~2.7us per switch)."""

    def insert_act_table_loads(self):
        import bass_rust as _bass_rust
        from concourse.hw_specs import get_activation_tables
        has_activation = any(
            isinstance(i, mybir.InstActivation)
            for b in self.main_func.blocks
            for i in b.instructions
        )
        if not has_activation:
            return
        # act_func_set_id is positional: keep list order, but hide every
        # set except the combined one so the pass can only pick it.
        items = [
            (nm, fns if nm == "natural_log_exp_and_others" else set())
            for nm, fns in get_activation_tables(self.m.arch).items()
        ]
        _bass_rust.insert_act_table_loads(self, items)


B, S, HID, H, KVH, D = 2, 2048, 2048, 16, 8, 128
G = H // KVH              # q heads per kv head (= per core)
EPS = 1e-6
SCALE = D ** -0.5
CH = 512                  # token chunk (proj phase)
NCORES = 8


def build_nc(S_=S):
    """Build the single-core SPMD program (identical on all 8 cores)."""
    HC = HID // 128           # hid chunks
    N = B * S_                # total tokens
    SK = S_ // 128            # k-tiles per batch
    NP = S_ // 256            # q-tile pairs per batch
    CPB = S_ // CH            # token chunks per batch
    NT = CH // 128            # 128-tok tiles per chunk

    nc = _Bacc(None)
    nc._phase_marks = []
    _mark = lambda s: nc._phase_marks.append((s, nc.next_id()))

    hsT_d = nc.dram_tensor("hsT", [HID, N], BF, kind="ExternalInput")
    wq_d = nc.dram_tensor("wq", [HC, 128, 512], BF, kind="ExternalInput")
    wk_d = nc.dram_tensor("wk", [HC, 128, 128], BF, kind="ExternalInput")
    wv_d = nc.dram_tensor("wv", [HC, 128, 128], BF, kind="ExternalInput")
    wo_d = nc.dram_tensor("wo", [G, 128, HID], BF, kind="ExternalInput")
    cq_d = nc.dram_tensor("cosq", [128, S_], BF, kind="ExternalInput")
    sq_d = nc.dram_tensor("sinq", [128, S_], BF, kind="ExternalInput")
    ck_d = nc.dram_tensor("cosk", [128, S_], BF, kind="ExternalInput")
    sk_d = nc.dram_tensor("sink", [128, S_], BF, kind="ExternalInput")
    id_d = nc.dram_tensor("ident", [128, 128], BF, kind="ExternalInput")
    o1_d = nc.dram_tensor("ones1", [128, 1], BF, kind="ExternalInput")
    ob_d = nc.dram_tensor("onesb", [1, 128], BF, kind="ExternalInput")
    o128_d = nc.dram_tensor("ones128", [128, 128], BF, kind="ExternalInput")
    ma_d = nc.dram_tensor("maska", [128, 512], BF, kind="ExternalInput")
    mb_d = nc.dram_tensor("maskb", [128, 512], BF, kind="ExternalInput")
    out_d = nc.dram_tensor("out", [N, HID], BF, kind="ExternalOutput")

    with tile.TileContext(nc) as tc, ExitStack() as ctx:
        cpool = ctx.enter_context(tc.tile_pool(name="consts", bufs=1))

        wq_s = cpool.tile([128, HC, 512], BF)
        wk_s = cpool.tile([128, HC, 128], BF)
        wv_s = cpool.tile([128, HC, 128], BF)
        wo_s = cpool.tile([128, G, HID], BF)
        cq_s = cpool.tile([128, S_], BF)
        sq_s = cpool.tile([128, S_], BF)
        ck_s = cpool.tile([128, S_], BF)
        sk_s = cpool.tile([128, S_], BF)
        id_s = cpool.tile([128, 128], BF)
        o1_s = cpool.tile([128, 1], BF)
        ob_s = cpool.tile([1, 128], BF)
        o128_s = cpool.tile([128, 128], BF)
        ma_s = cpool.tile([128, 512], BF)
        mb_s = cpool.tile([128, 512], BF)
        epsb = cpool.tile([128, 1], F32)
        oneb = cpool.tile([128, 1], F32)
        nc.vector.memset(epsb[:], EPS)
        nc.vector.memset(oneb[:], 1.0)

        # per-chunk weight loads so the first projection matmuls unblock
        # as soon as their own W chunk lands (not the whole 2MB tensor)
        for c in range(HC):
            nc.sync.dma_start(wq_s[:, c, :], wq_d[c])
            nc.scalar.dma_start(wk_s[:, c, :], wk_d[c])
            nc.scalar.dma_start(wv_s[:, c, :], wv_d[c])
        nc.sync.dma_start(wo_s[:], wo_d[:].rearrange("c p f -> p c f"))
        for dst, src in ((cq_s, cq_d), (sq_s, sq_d), (ck_s, ck_d), (sk_s, sk_d),
                         (id_s, id_d), (o1_s, o1_d), (ob_s, ob_d),
                         (o128_s, o128_d), (ma_s, ma_d), (mb_s, mb_d)):
            nc.sync.dma_start(dst[:], src[:])

        _mark('consts')
        # persistent activations (feature-major: [D, ...tok])
        qtb = cpool.tile([128, B, SK, G, 128], BF)   # rope'd+normed q
        ktb = cpool.tile([128, B, SK, 128], BF)      # rope'd+normed k
        vtb = cpool.tile([128, N], BF)               # v, feature-major
        vb = cpool.tile([128, B, SK, 128], BF)       # v, token-major
        gtb = cpool.tile([128, B, SK, G, 128], BF)   # sigmoid(gate)

        # ---------------- phase 1: projections ----------------
        hsT_v = hsT_d[:].rearrange("(c p) n -> c p n", p=128)
        with (
            tc.tile_pool(name="hst", bufs=2) as hstp,
            tc.tile_pool(name="projps", bufs=6, space="PSUM") as projps,
            tc.tile_pool(name="ssps", bufs=1, space="PSUM") as ssps,
            tc.tile_pool(name="auxps", bufs=1, space="PSUM") as auxps,
            tc.tile_pool(name="pwork", bufs=3) as pwork,
        ):
            for b in range(B):
                for cc in range(CPB):
                    t0 = b * S_ + cc * CH     # global token start
                    p0 = cc * CH              # position start (within batch)
                    ht = hstp.tile([128, HC, CH], BF, tag="hst")
                    # finer sub-DMAs for the very first chunk so the first
                    # projection matmuls unblock as early as possible
                    step = 2 if (b == 0 and cc == 0) else 4
                    for c4 in range(0, HC, step):
                        nc.gpsimd.dma_start(
                            ht[:, c4:c4 + step, :],
                            hsT_v[c4:c4 + step, :, t0:t0 + CH].rearrange(
                                "c p f -> p c f"))
                    hts = [ht[:, c, :] for c in range(HC)]

                    psq0 = projps.tile([128, CH], F32, tag="pp")
                    psq1 = projps.tile([128, CH], F32, tag="pp")
                    psk = projps.tile([128, CH], F32, tag="pp")
                    psv = projps.tile([128, CH], F32, tag="pp")
                    psg0 = projps.tile([128, CH], F32, tag="pp")
                    psg1 = projps.tile([128, CH], F32, tag="pp")
                    for c in range(HC):
                        st, sp = c == 0, c == HC - 1
                        nc.tensor.matmul(psq0[:], wq_s[:, c, 0:128], hts[c],
                                         start=st, stop=sp)
                        nc.tensor.matmul(psq1[:], wq_s[:, c, 128:256], hts[c],
                                         start=st, stop=sp)
                        nc.tensor.matmul(psk[:], wk_s[:, c, :], hts[c],
                                         start=st, stop=sp)
                        nc.tensor.matmul(psv[:], wv_s[:, c, :], hts[c],
                                         start=st, stop=sp)
                        nc.tensor.matmul(psg0[:], wq_s[:, c, 256:384], hts[c],
                                         start=st, stop=sp)
                        nc.tensor.matmul(psg1[:], wq_s[:, c, 384:512], hts[c],
                                         start=st, stop=sp)

                    ti0 = cc * NT
                    # RMSNorm + RoPE for q heads and k
                    blocks = [
                        (psq0, cq_s, sq_s, qtb[:, b, ti0:ti0 + NT, 0, :]),
                        (psq1, cq_s, sq_s, qtb[:, b, ti0:ti0 + NT, 1, :]),
                        (psk, ck_s, sk_s, ktb[:, b, ti0:ti0 + NT, :]),
                    ]
                    for psx, ctab, stab, dest in blocks:
                        xu = pwork.tile([128, CH], BF, tag="xu")
                        nc.scalar.copy(xu[:], psx[:])
                        xsq = pwork.tile([128, CH], BF, tag="xsq")
                        nc.vector.tensor_mul(xsq[:], xu[:], xu[:])
                        ssp = pwork.tile([1, CH], F32, tag="ssp")
                        nc.gpsimd.tensor_reduce(ssp[:], xsq[:],
                                                mybir.AxisListType.C,
                                                mybir.AluOpType.add)
                        ssl = pwork.tile([1, CH], F32, tag="ssl")
                        nc.scalar.activation(ssl[:], ssp[:], AF.Ln,
                                             bias=epsb[:1], scale=1.0 / D)
                        rsts = pwork.tile([1, CH], BF, tag="rsts")
                        nc.scalar.activation(rsts[:], ssl[:], AF.Exp, scale=-0.5)
                        rstdB = auxps.tile([128, CH], F32, tag="aux")
                        nc.tensor.matmul(rstdB[:], ob_s[:], rsts[:])
                        t1 = pwork.tile([128, CH], BF, tag="t1")
                        nc.vector.tensor_mul(t1[:], xu[:], ctab[:, p0:p0 + CH])
                        xrot = pwork.tile([128, CH], BF, tag="xrot")
                        nc.vector.tensor_copy(xrot[0:64, :], xu[64:128, :])
                        nc.vector.tensor_copy(xrot[64:128, :], xu[0:64, :])
                        t2 = pwork.tile([128, CH], BF, tag="t2")
                        nc.vector.tensor_mul(t2[:], xrot[:],
                                             stab[:, p0:p0 + CH])
                        nc.vector.tensor_add(t1[:], t1[:], t2[:])
                        nc.vector.tensor_mul(dest, t1[:], rstdB[:])

                    # v: stash feature-major (transposed later)
                    nc.scalar.copy(vtb[:, t0:t0 + CH], psv[:])

                    # gates: sigmoid(g) = exp(-ln(1 + exp(-g)))
                    for hh, psg in ((0, psg0), (1, psg1)):
                        e1 = pwork.tile([128, CH], BF, tag="e1")
                        nc.scalar.activation(e1[:], psg[:], AF.Exp, scale=-1.0)
                        l1 = pwork.tile([128, CH], F32, tag="l1")
                        nc.scalar.activation(l1[:], e1[:], AF.Ln, bias=oneb[:])
                        nc.scalar.activation(gtb[:, b, ti0:ti0 + NT, hh, :],
                                             l1[:], AF.Exp, scale=-1.0)
                    _mark(f'proj b{b}c{cc}')

            # ---------------- phase 2: V -> token-major ----------------
            for b in range(B):
                for j4 in range(0, SK, 4):
                    vt_ps = auxps.tile([128, 512], BF, tag="aux",
                                       name="vt_ps")
                    for jj in range(4):
                        j = j4 + jj
                        nc.tensor.transpose(
                            vt_ps[:, jj * 128:(jj + 1) * 128],
                            vtb[:, b * S_ + j * 128: b * S_ + (j + 1) * 128],
                            id_s[:])
                    nc.scalar.copy(vb[:, b, j4:j4 + 4, :], vt_ps[:])

        _mark('vtrans')
        # ---------------- phase 3+4: attention + gating + Wo ----------------
        with (
            tc.tile_pool(name="scps", bufs=2, space="PSUM") as scps,
            tc.tile_pool(name="pvps", bufs=2, space="PSUM") as pvps,
            tc.tile_pool(name="sumps", bufs=2, space="PSUM") as sumps,
            tc.tile_pool(name="wops", bufs=2, space="PSUM") as wops,
            tc.tile_pool(name="probsp", bufs=6) as probsp,
            tc.tile_pool(name="awork", bufs=3) as awork,
        ):
            def wo_proj(b, i0, gated):
                # gating result of pair (b, i0//2) -> Wo row-shard -> DRAM
                for it in range(2):
                    trow = b * S_ + (i0 + it) * 128
                    osb = awork.tile([128, HID], BF, tag="osb")
                    for oc in range(HID // 512):
                        wop = wops.tile([128, 512], F32, tag="wo")
                        nc.tensor.matmul(
                            wop[:], gated[:, it * 256:it * 256 + 128],
                            wo_s[:, 0, oc * 512:(oc + 1) * 512],
                            start=True, stop=False)
                        nc.tensor.matmul(
                            wop[:], gated[:, it * 256 + 128:it * 256 + 256],
                            wo_s[:, 1, oc * 512:(oc + 1) * 512],
                            start=False, stop=True)
                        nc.vector.tensor_copy(
                            osb[:, oc * 512:(oc + 1) * 512], wop[:])
                    nc.gpsimd.dma_start(out_d[trow:trow + 128, :], osb[:])

            # Two-pair interleaved attention: pairs (2g, 2g+1) advance their
            # j-loops together (independent psum accumulators), so the PE has
            # ~6 matmuls in flight per j step to hide each exp's latency.
            # The Wo projection of the previous group is emitted after the
            # current group's attention as additional filler.
            def attn_pair(st, j):
                b, i0, jmax, pv, smp = st
                scp = scps.tile([128, 512], F32, tag="sc")
                nc.tensor.matmul(scp[:], ktb[:, b, j, :],
                                 qtb[:, b, i0:i0 + 2, :, :])
                probs = probsp.tile([128, 512], BF, tag="probs")
                nc.scalar.activation(probs[:], scp[:], AF.Exp)
                if j == i0:
                    nc.vector.tensor_mul(probs[:], probs[:], ma_s[:])
                elif j == jmax:
                    nc.vector.tensor_mul(probs[:], probs[:], mb_s[:])
                nc.tensor.matmul(pv[:], vb[:, b, j, :], probs[:],
                                 start=(j == 0), stop=(j == jmax))
                nc.tensor.matmul(smp[:], o128_s[:], probs[:],
                                 start=(j == 0), stop=(j == jmax))

            def gate_pair(st):
                b, i0, jmax, pv, smp = st
                lsb = awork.tile([128, 512], F32, tag="lsb")
                nc.scalar.activation(lsb[:], smp[:], AF.Ln)
                rsb = awork.tile([128, 512], F32, tag="rsb")
                nc.scalar.activation(rsb[:], lsb[:], AF.Exp, scale=-1.0)
                tmp = awork.tile([128, 512], BF, tag="tmp")
                nc.vector.tensor_mul(tmp[:], pv[:], rsb[:])
                gated = probsp.tile([128, 512], BF, tag="gated")
                nc.vector.tensor_mul(gated[:], tmp[:],
                                     gtb[:, b, i0:i0 + 2, :, :])
                return (b, i0, gated)

            pending = []
            for b in range(B):
                for pA in range(0, NP, 2):
                    pB = pA + 1
                    sts = []
                    for p in (pA, pB):
                        i0 = 2 * p
                        sts.append((b, i0, i0 + 1,
                                    pvps.tile([128, 512], F32, tag="pv",
                                              name="pv"),
                                    sumps.tile([128, 512], F32, tag="sm",
                                               name="sm")))
                    stA, stB = sts
                    done = []
                    for j in range(stB[2] + 1):
                        if j <= stA[2]:
                            attn_pair(stA, j)
                        attn_pair(stB, j)
                        if j == stA[2]:
                            done.append(gate_pair(stA))
                            if pending:
                                wo_proj(*pending.pop(0))
                    done.append(gate_pair(stB))
                    _mark(f'attn b{b}g{pA//2}')
                    for pend in pending:
                        wo_proj(*pend)
                    pending = done
            for pend in pending:
                wo_proj(*pend)
    nc.compile()
    return nc


def prep_inputs(hidden_states, cos, sin, Wq, Wk, Wv, Wo, q_norm_w, k_norm_w,
                S_=S):
    """Host-side sharding + layout prep. Returns in_maps for 8 cores."""
    N = B * S_
    hsT = np.ascontiguousarray(
        hidden_states.reshape(N, HID).T).astype(BF16)

    cos0 = np.asarray(cos[0], np.float32)   # [S_, D] (identical across batch)
    sin0 = np.asarray(sin[0], np.float32)
    qw = np.asarray(q_norm_w, np.float32)
    kw = np.asarray(k_norm_w, np.float32)
    sign = np.where(np.arange(D) < 64, -1.0, 1.0).astype(np.float32)
    shift = (np.arange(D) + 64) % D

    cosq = np.ascontiguousarray(cos0.T * qw[:, None] * SCALE).astype(BF16)
    sinq = np.ascontiguousarray(
        sin0.T * (sign * qw[shift])[:, None] * SCALE).astype(BF16)
    cosk = np.ascontiguousarray(cos0.T * kw[:, None]).astype(BF16)
    sink = np.ascontiguousarray(
        sin0.T * (sign * kw[shift])[:, None]).astype(BF16)

    tri = (np.arange(128)[:, None] <= np.arange(128)[None, :])
    onesq = np.ones((128, 128), np.float32)
    maska = np.concatenate([tri, tri, onesq, onesq], axis=1).astype(BF16)
    maskb = np.concatenate([0 * onesq, 0 * onesq, tri, tri],
                           axis=1).astype(BF16)
    ident = np.eye(128, dtype=BF16)
    ones1 = np.ones((128, 1), BF16)
    onesb = np.ones((1, 128), BF16)
    ones128 = np.ones((128, 128), BF16)

    HC = HID // 128
    in_maps = []
    for d in range(NCORES):
        h0, h1 = G * d, G * d + 1
        q0 = Wq[:, h0 * 2 * D: h0 * 2 * D + D]
        g0 = Wq[:, h0 * 2 * D + D: (h0 + 1) * 2 * D]
        q1 = Wq[:, h1 * 2 * D: h1 * 2 * D + D]
        g1 = Wq[:, h1 * 2 * D + D: (h1 + 1) * 2 * D]
        wq_c = np.concatenate([q0, q1, g0, g1], axis=1)      # [HID, 512]
        wq_a = np.ascontiguousarray(wq_c).astype(BF16).reshape(HC, 128, 512)
        wk_a = np.ascontiguousarray(
            Wk[:, d * D:(d + 1) * D]).astype(BF16).reshape(HC, 128, 128)
        wv_a = np.ascontiguousarray(
            Wv[:, d * D:(d + 1) * D]).astype(BF16).reshape(HC, 128, 128)
        wo_a = np.ascontiguousarray(
            Wo[d * G * D:(d + 1) * G * D, :]).astype(BF16).reshape(G, 128, HID)
        in_maps.append({
            "hsT": hsT, "wq": wq_a, "wk": wk_a, "wv": wv_a, "wo": wo_a,
            "cosq": cosq, "sinq": sinq, "cosk": cosk, "sink": sink,
            "ident": ident, "ones1": ones1, "onesb": onesb,
            "ones128": ones128, "maska": maska, "maskb": maskb,
        })
    return in_maps


_NC_CACHE = {}
_RUNNER_CACHE = {}


def _get_nc(S_=S):
    if S_ not in _NC_CACHE:
        _NC_CACHE[S_] = build_nc(S_)
    return _NC_CACHE[S_]


def _get_runner(S_=S):
    """Build a cached jitted 8-core executable.

    Mirrors concourse.bass2jax.run_bass_via_pjrt's multi-core path, but
    keeps the jitted function (and device-resident output placeholders)
    so repeated calls don't re-trace/re-compile, and so the executable
    can be timed in a steady-state loop.
    """
    if S_ in _RUNNER_CACHE:
        return _RUNNER_CACHE[S_]
    import jax
    from jax.experimental.shard_map import shard_map
    from jax.sharding import Mesh, PartitionSpec
    from concourse import bass2jax, mybir as _mybir
    bass2jax.install_neuronx_cc_hook()

    nc = _get_nc(S_)
    assert nc.dbg_addr is None
    pid_name = (nc.partition_id_tensor.name
                if nc.partition_id_tensor is not None else None)

    in_names, out_names, out_avals = [], [], []
    for alloc in nc.m.functions[0].allocations:
        if not isinstance(alloc, _mybir.MemoryLocationSet):
            continue
        name = alloc.memorylocations[0].name
        if alloc.kind == "ExternalInput":
            if name != pid_name:
                in_names.append(name)
        elif alloc.kind == "ExternalOutput":
            out_names.append(name)
            out_avals.append(jax.core.ShapedArray(
                tuple(alloc.tensor_shape), _mybir.dt.np(alloc.dtype)))
    n_params = len(in_names)
    all_names = in_names + out_names
    if pid_name is not None:
        all_names = all_names + [pid_name]

    def _body(*args):
        operands = list(args)
        if pid_name is not None:
            operands.append(bass2jax.partition_id_tensor())
        outs = bass2jax._bass_exec_p.bind(
            *operands,
            out_avals=tuple(out_avals),
            in_names=tuple(all_names),
            out_names=tuple(out_names),
            lowering_input_output_aliases=(),
            sim_require_finite=True,
            sim_require_nnan=True,
            nc=nc,
        )
        return tuple(outs)

    devices = jax.devices()[:NCORES]
    mesh = Mesh(np.asarray(devices), ("core",))
    nin = n_params + len(out_names)
    sharded = jax.jit(
        shard_map(_body, mesh=mesh,
                  in_specs=(PartitionSpec("core"),) * nin,
                  out_specs=(PartitionSpec("core"),) * len(out_names),
                  check_rep=False),
        keep_unused=True,
    )
    zeros = [np.zeros((NCORES * a.shape[0], *a.shape[1:]), a.dtype)
             for a in out_avals]
    zeros_dev = [jax.device_put(z) for z in zeros]

    def run(in_maps):
        concat_in = [
            np.concatenate([np.asarray(m[nm]) for m in in_maps], axis=0)
            for nm in in_names
        ]
        outs = sharded(*concat_in, *zeros_dev)
        return {nm: np.asarray(outs[i]) for i, nm in enumerate(out_names)}

    def run_prepared(dev_args):
        return sharded(*dev_args, *zeros_dev)

    def prepare(in_maps):
        return [
            jax.device_put(np.concatenate(
                [np.asarray(m[nm]) for m in in_maps], axis=0))
            for nm in in_names
        ]

    r = {"run": run, "prepare": prepare, "run_prepared": run_prepared,
         "out_names": out_names, "out_avals": out_avals}
    _RUNNER_CACHE[S_] = r
    return r


def kernel(hidden_states, cos, sin, Wq, Wk, Wv, Wo, q_norm_w, k_norm_w):
    in_maps = prep_inputs(hidden_states, cos, sin, Wq, Wk, Wv, Wo,
                          q_norm_w, k_norm_w)
    runner = _get_runner()
    outs = runner["run"](in_maps)
    full = outs["out"].reshape(NCORES, B * S, HID)
    acc = full.astype(np.float32).sum(axis=0)
    return acc.reshape(B, S, HID)



# revision 3
# speedup vs baseline: 1.0342x; 1.0342x over previous
"""Qwen-style GQA full attention (B=2, S=2048, HID=2048, H=16, KVH=8, D=128)
on 8 trn2 NeuronCores — v2: hi-lo fp8 DoubleRow matmuls.

Sharding: tensor-parallel across head groups (core d owns kv-head d and its
two query heads). Each core emits a partial [B*S, HID] via its Wo row block;
the host sums the 8 partials in f32.

Numerics: projections and Wo run as error-compensated fp8 ("hi-lo"): each
operand x is pre-scaled into e4m3's normal range and split x ~= hi + lo
(both e4m3); y = hi*Wh + lo*Wh + hi*Wl (3 DoubleRow matmuls, K=256 each,
the lo*lo term ~0.03% is dropped). This measures *better* than bf16
(0.11% vs 0.23% on the projection GEMM) at 1.33x bf16 matmul throughput
(DoubleRow streams 2 fp8 rows/cycle). Attention (QK/PV/denominator-sum)
stays bf16: with random weights the softmax is diffuse, so per-element
quantization noise in q/k/v/probs reaches the output at full strength
(fp8 there measurably busts the 2e-2 budget).

Performance notes (sim cost model):
  - PE sequencer dispatch is ~84ns per matmul (ldweights+matmult); small
    matmuls are sequencer-bound, so every DR matmul moves the full 1024
    fp8 elements/row (512 output cols) and V is projected feature-major
    (512-wide moving) then PE-transposed, instead of token-major 128-wide.
  - GPSIMD (Pool) is Q7 software: ~0.42 efficiency + 95ns launch, and it
    cannot touch PSUM; only SBUF-side work with slack goes there.
  - Hot-path DMAs ride the SP and ACT hardware queues; bulk constants go
    on the Pool queue after the first chunk's loads (the cost model
    serializes all transfers through one DMA device, so early ordering
    decides when the first matmuls can start).
  - RMSNorm sum-of-squares is a ones-vector PE matmul into PSUM (the Pool
    C-axis reduce is Q7 software and was the chunk-rate limiter); the rstd
    broadcast matmuls sit after the later gate blocks so Ln/Exp has
    finished by the time the PE reaches them.
  - sigmoid(g) = 1/(1+exp(-g)): ACT exp + DVE add/reciprocal, keeping the
    ACT table set at {Ln, Exp, Copy} (one table load, no thrash).
"""

import numpy as np
import ml_dtypes

import concourse.bass as bass
import concourse.tile as tile
from concourse import bacc, mybir
from contextlib import ExitStack

BF16 = ml_dtypes.bfloat16
E4NP = ml_dtypes.float8_e4m3
F32 = mybir.dt.float32
BF = mybir.dt.bfloat16
E4 = mybir.dt.float8e4
AF = mybir.ActivationFunctionType
DR = mybir.MatmulPerfMode.DoubleRow
ALU = mybir.AluOpType


class _Bacc(bacc.Bacc):
    """Pin the combined Ln+Exp activation table set (see module docstring)."""

    def insert_act_table_loads(self):
        import bass_rust as _bass_rust
        from concourse.hw_specs import get_activation_tables
        has_activation = any(
            isinstance(i, mybir.InstActivation)
            for b in self.main_func.blocks
            for i in b.instructions
        )
        if not has_activation:
            return
        items = [
            (nm, fns if nm == "natural_log_exp_and_others" else set())
            for nm, fns in get_activation_tables(self.m.arch).items()
        ]
        _bass_rust.insert_act_table_loads(self, items)


B, S, HID, H, KVH, D = 2, 2048, 2048, 16, 8, 128
G = H // KVH
EPS = 1e-6
SCALE = D ** -0.5
CH = 512
NCORES = 8

SX = 16.0        # hidden-state fp8 pre-scale
SW = 128.0       # weight fp8 pre-scale
SG = 32.0        # gated-output fp8 pre-scale
IXW = 1.0 / (SX * SW)
IGW = 1.0 / (SG * SW)

# packed projection-weight column layout: q0 q1 k v g0 g1 (128 each)
COLS = {"q0": 0, "q1": 128, "k": 256, "v": 384, "g0": 512, "g1": 640}


def build_nc(S_=S):
    HC = HID // 128
    N = B * S_
    SK = S_ // 128
    NP = S_ // 256
    CPB = S_ // CH
    NT = CH // 128
    NKP = HC // 2

    nc = _Bacc(None)

    hsthi_d = nc.dram_tensor("hsthi", [HC, 128, N], E4, kind="ExternalInput")
    hstlo_d = nc.dram_tensor("hstlo", [HC, 128, N], E4, kind="ExternalInput")
    whi_d = nc.dram_tensor("whi", [HC, 128, 768], E4, kind="ExternalInput")
    wlo_d = nc.dram_tensor("wlo", [HC, 128, 768], E4, kind="ExternalInput")
    wohi_d = nc.dram_tensor("wohi", [G, 128, HID], E4, kind="ExternalInput")
    wolo_d = nc.dram_tensor("wolo", [G, 128, HID], E4, kind="ExternalInput")
    cq2_d = nc.dram_tensor("cosq2", [128, 2, S_], BF, kind="ExternalInput")
    sq2_d = nc.dram_tensor("sinq2", [128, 2, S_], BF, kind="ExternalInput")
    ck_d = nc.dram_tensor("cosk", [128, S_], BF, kind="ExternalInput")
    sk_d = nc.dram_tensor("sink", [128, S_], BF, kind="ExternalInput")
    tri_d = nc.dram_tensor("tri2", [128, 2, 256], BF, kind="ExternalInput")
    id_d = nc.dram_tensor("ident", [128, 128], BF, kind="ExternalInput")
    out_d = nc.dram_tensor("out", [N, HID], BF, kind="ExternalOutput")

    with tile.TileContext(nc) as tc, ExitStack() as ctx:
        cpool = ctx.enter_context(tc.tile_pool(name="consts", bufs=1))

        whi_s = cpool.tile([128, HC, 768], E4)
        wlo_s = cpool.tile([128, HC, 768], E4)
        wohi_s = cpool.tile([128, G, HID], E4)
        wolo_s = cpool.tile([128, G, HID], E4)
        cq2_s = cpool.tile([128, 2, S_], BF)
        sq2_s = cpool.tile([128, 2, S_], BF)
        ck_s = cpool.tile([128, S_], BF)
        sk_s = cpool.tile([128, S_], BF)
        tri_s = cpool.tile([128, 2, 256], BF)
        id_s = cpool.tile([128, 128], BF)
        ones_s = cpool.tile([128, 128], BF)
        o1_s = cpool.tile([128, 1], BF)
        ob_s = cpool.tile([1, 128], BF)
        epsb = cpool.tile([1, 1], F32)
        nc.vector.memset(ones_s[:], 1.0)
        nc.vector.memset(o1_s[:], 1.0)
        nc.vector.memset(ob_s[:], 1.0)
        nc.vector.memset(epsb[:], EPS)

        # weight loads interleaved on ACT (the first matmuls need whi+wlo);
        # tables/Wo weights ride the Pool queue (slack early, needed later)
        whi_v = whi_d[:].rearrange("c p f -> p c f")
        wlo_v = wlo_d[:].rearrange("c p f -> p c f")
        nc.scalar.dma_start(whi_s[:, 0:4, :], whi_v[:, 0:4, :])
        nc.scalar.dma_start(wlo_s[:, 0:4, :], wlo_v[:, 0:4, :])

        # persistent activations
        qtb = cpool.tile([128, B, SK, G, 128], BF)   # rope'd+normed q (feat-major)
        ktb = cpool.tile([128, B, SK, 128], BF)      # rope'd+normed k (feat-major)
        vtb = cpool.tile([128, N], BF)               # v feature-major staging
        vb = cpool.tile([128, B, SK, 128], BF)       # v token-major
        gtb = cpool.tile([128, B, SK, G, 128], F32)  # sigmoid(gate)

        # ---------------- phase 1: projections (hi-lo fp8 DR) ----------------
        # Blocks are emitted sequentially (all 24 accumulating matmuls of one
        # output block, then its consumers) so each PSUM bank is freed ~13us
        # before the next chunk needs it, and consumer work spreads evenly.
        with (
            tc.tile_pool(name="hst", bufs=2) as hstp,
            tc.tile_pool(name="qgps", bufs=3, space="PSUM") as qgps,
            tc.tile_pool(name="auxps", bufs=2, space="PSUM") as auxps,
            tc.tile_pool(name="ssps", bufs=3, space="PSUM") as ssps,
            tc.tile_pool(name="pwork", bufs=2) as pwork,
            tc.tile_pool(name="pw1", bufs=1) as pw1,
        ):
            for b in range(B):
                for cc in range(CPB):
                    t0 = b * S_ + cc * CH
                    p0 = cc * CH
                    ti0 = cc * NT
                    first_chunk = b == 0 and cc == 0
                    hh = hstp.tile([128, HC, CH], E4, tag="hh")
                    hl = hstp.tile([128, HC, CH], E4, tag="hl")
                    if first_chunk:
                        for c4 in range(4, HC, 4):
                            nc.scalar.dma_start(whi_s[:, c4:c4 + 4, :],
                                                whi_v[:, c4:c4 + 4, :])
                            nc.scalar.dma_start(wlo_s[:, c4:c4 + 4, :],
                                                wlo_v[:, c4:c4 + 4, :])
                        # tables after chunk-0 weights: needed only by the
                        # first rope/mask consumers, keep them off the
                        # critical early DMA window
                        for dst, srct in ((cq2_s, cq2_d), (sq2_s, sq2_d),
                                          (ck_s, ck_d), (sk_s, sk_d),
                                          (tri_s, tri_d), (id_s, id_d)):
                            nc.gpsimd.dma_start(dst[:], srct[:])
                        nc.gpsimd.dma_start(
                            wohi_s[:], wohi_d[:].rearrange("c p f -> p c f"))
                        nc.gpsimd.dma_start(
                            wolo_s[:], wolo_d[:].rearrange("c p f -> p c f"))
                    step = 2 if first_chunk else 8
                    for c4 in range(0, HC, step):
                        nc.sync.dma_start(
                            hh[:, c4:c4 + step, :],
                            hsthi_d[c4:c4 + step, :, t0:t0 + CH].rearrange(
                                "c p f -> p c f"))
                        nc.sync.dma_start(
                            hl[:, c4:c4 + step, :],
                            hstlo_d[c4:c4 + step, :, t0:t0 + CH].rearrange(
                                "c p f -> p c f"))

                    xus = {}
                    ssts = {}
                    rsts = pw1.tile([1, 3, CH], BF, tag="rsts")
                    e1 = pw1.tile([128, 2, CH], BF, tag="e1")
                    s12s = {}

                    def run_blocks(specs):
                        # specs: [(nm, n_prods)]; for the DMA-bound first
                        # chunk the blocks advance kp-inner together so
                        # matmuls track ht chunk arrival instead of waiting
                        # for the whole tensor
                        pss, prodss = [], []
                        for nm, n_prods in specs:
                            pss.append(qgps.tile([128, CH], F32, tag="pp",
                                                 name=f"ps_{nm}"))
                            prodss.append(
                                [(whi_s, hh), (whi_s, hl),
                                 (wlo_s, hh)][:n_prods])
                        for kp in range(NKP):
                            c = 2 * kp
                            for (nm, _), ps, prods in zip(specs, pss, prodss):
                                col0 = COLS[nm]
                                for pi, (wsrc, hsrc) in enumerate(prods):
                                    nc.tensor.matmul(
                                        ps[:],
                                        wsrc[:, c:c + 2, col0:col0 + 128],
                                        hsrc[:, c:c + 2, :],
                                        start=kp == 0 and pi == 0,
                                        stop=(kp == NKP - 1
                                              and pi == len(prods) - 1),
                                        perf_mode=DR)
                        return pss

                    def run_block(nm, n_prods):
                        return run_blocks([(nm, n_prods)])[0]

                    def norm_stats(nm, i, ps):
                        xu = pwork.tile([128, CH], BF, tag=f"xu_{nm}",
                                        name="xu")
                        nc.scalar.activation(xu[:], ps[:], AF.Copy, scale=IXW)
                        xus[nm] = xu
                        xsq = pw1.tile([128, CH], BF, tag=f"xsq_{nm}",
                                       name="xsq")
                        nc.vector.tensor_mul(xsq[:], xu[:], xu[:])
                        ssp = ssps.tile([1, CH], F32, tag="ss", name="ssp")
                        nc.tensor.matmul(ssp[:], o1_s[:], xsq[:])
                        ssts[nm] = ssp

                    def rope_sum(nm, i, ctab, stab, cidx):
                        # 2-input SBUF ops need equal base partitions, so the
                        # half-rotation is a pair of 1-input copies first.
                        xu = xus[nm]
                        t1 = pw1.tile([128, CH], BF, tag=f"t1_{nm}", name="t1")
                        nc.vector.tensor_mul(t1[:], xu[:],
                                             ctab[:, cidx, p0:p0 + CH]
                                             if cidx is not None
                                             else ctab[:, p0:p0 + CH])
                        xr = pw1.tile([128, CH], BF, tag=f"xr_{nm}", name="xr")
                        nc.vector.tensor_copy(xr[0:64, :], xu[64:128, :])
                        nc.vector.tensor_copy(xr[64:128, :], xu[0:64, :])
                        t2 = pw1.tile([128, CH], BF, tag=f"t2_{nm}", name="t2")
                        sv = (stab[:, cidx, p0:p0 + CH] if cidx is not None
                              else stab[:, p0:p0 + CH])
                        nc.gpsimd.tensor_mul(t2[:], xr[:], sv)
                        s12 = pw1.tile([128, CH], BF, tag=f"s12_{nm}",
                                       name="s12")
                        nc.vector.tensor_add(s12[:], t1[:], t2[:])
                        s12s[nm] = s12

                    def emit_bc(i, s12, dest):
                        rstdB = auxps.tile([128, CH], F32, tag="aux",
                                           name="rstdB")
                        nc.tensor.matmul(rstdB[:], ob_s[:], rsts[0:1, i, :])
                        nc.vector.scalar_tensor_tensor(dest, s12[:], 1.0,
                                                       rstdB[:],
                                                       ALU.mult, ALU.mult)

                    tabs = {"q0": (cq2_s, sq2_s, 0), "q1": (cq2_s, sq2_s, 1),
                            "k": (ck_s, sk_s, None)}
                    names3 = ("q0", "q1", "k")

                    def qcons(nm):
                        i = names3.index(nm)
                        norm_stats(nm, i, pss[nm])
                        rope_sum(nm, i, *tabs[nm])

                    def ln_chain():
                        lnv = pw1.tile([1, 3, CH], BF, tag="lnv")
                        for i, nm in enumerate(names3):
                            nc.scalar.activation(lnv[:, i, :], ssts[nm][:],
                                                 AF.Ln, bias=epsb[:],
                                                 scale=1.0 / D)
                        nc.scalar.activation(rsts[:], lnv[:], AF.Exp,
                                             scale=-0.5)

                    def vcons():
                        nc.scalar.activation(vtb[:, t0:t0 + CH], pss["v"][:],
                                             AF.Copy, scale=IXW)

                    def gcons(h):
                        nc.scalar.activation(e1[:, h, :],
                                             pss["g0" if h == 0 else "g1"][:],
                                             AF.Exp, scale=-IXW)

                    pss = {}
                    if first_chunk:
                        # DMA-bound: advance pairs of blocks kp-inner so the
                        # PE tracks ht chunk arrival
                        for grp in ([("q0", 3), ("q1", 3)],
                                    [("k", 3), ("v", 3)],
                                    [("g0", 2), ("g1", 2)]):
                            res = run_blocks(grp)
                            pss.update({nm: ps for (nm, _), ps
                                        in zip(grp, res)})
                            if grp[0][0] == "q0":
                                qcons("q0")
                                qcons("q1")
                            elif grp[0][0] == "k":
                                qcons("k")
                                ln_chain()
                                vcons()
                            else:
                                gcons(0)
                                gcons(1)
                                emit_bc(0, s12s["q0"],
                                        qtb[:, b, ti0:ti0 + NT, 0, :])
                                emit_bc(1, s12s["q1"],
                                        qtb[:, b, ti0:ti0 + NT, 1, :])
                                emit_bc(2, s12s["k"],
                                        ktb[:, b, ti0:ti0 + NT, :])
                    else:
                        for nm in names3:
                            pss[nm] = run_block(nm, 3)
                            qcons(nm)
                        ln_chain()
                        pss["v"] = run_block("v", 3)
                        vcons()
                        pss["g0"] = run_block("g0", 2)
                        gcons(0)
                        emit_bc(0, s12s["q0"], qtb[:, b, ti0:ti0 + NT, 0, :])
                        emit_bc(1, s12s["q1"], qtb[:, b, ti0:ti0 + NT, 1, :])
                        pss["g1"] = run_block("g1", 2)
                        gcons(1)
                        emit_bc(2, s12s["k"], ktb[:, b, ti0:ti0 + NT, :])

                    # sigmoid: 1/(1+e1) -> gtb (f32)
                    a1 = pw1.tile([128, 2, CH], F32, tag="a1")
                    nc.vector.tensor_scalar_add(a1[:], e1[:], 1.0)
                    for h in range(G):
                        nc.vector.reciprocal(
                            gtb[:, b, ti0:ti0 + NT, h, :], a1[:, h, :])

                # V -> token-major for this batch (PE transposes)
                for j4 in range(0, SK, 4):
                    vt_ps = auxps.tile([128, 512], BF, tag="aux", name="vt")
                    for jj in range(4):
                        j = j4 + jj
                        nc.tensor.transpose(
                            vt_ps[:, jj * 128:(jj + 1) * 128],
                            vtb[:, b * S_ + j * 128:b * S_ + (j + 1) * 128],
                            id_s[:])
                    nc.vector.tensor_copy(vb[:, b, j4:j4 + 4, :], vt_ps[:])

        # ---------------- phase 2: attention + gating + Wo ----------------
        with (
            tc.tile_pool(name="scps", bufs=2, space="PSUM") as scps,
            tc.tile_pool(name="pvps", bufs=1, space="PSUM") as pvps,
            tc.tile_pool(name="sumps", bufs=1, space="PSUM") as sumps,
            tc.tile_pool(name="wops", bufs=2, space="PSUM") as wops,
            tc.tile_pool(name="probsp", bufs=4) as probsp,
            tc.tile_pool(name="awork", bufs=3) as awork,
        ):
            def wo_proj(b, i0, ghi, glo, split_store=False):
                osb = awork.tile([128, 2, HID], BF, tag="osb")
                trow = b * S_ + i0 * 128
                out_v = out_d[trow:trow + 256, :].rearrange(
                    "(a p) f -> p a f", p=128)
                for it in range(2):
                    for oc in range(4):
                        wop = wops.tile([128, 512], F32, tag="wo")
                        for pi, (gs, ws) in enumerate(
                                ((ghi, wohi_s), (glo, wohi_s), (ghi, wolo_s))):
                            nc.tensor.matmul(
                                wop[:], gs[:, it, :, :],
                                ws[:, :, oc * 512:(oc + 1) * 512],
                                start=pi == 0, stop=pi == 2, perf_mode=DR)
                        dst = osb[:, it, oc * 512:(oc + 1) * 512]
                        if oc % 2 == 0:
                            nc.vector.tensor_scalar_mul(dst, wop[:], IGW)
                        else:
                            nc.scalar.activation(dst, wop[:], AF.Copy,
                                                 scale=IGW)
                    if split_store:
                        q = nc.sync if it == 0 else nc.gpsimd
                        q.dma_start(out_v[:, it, :], osb[:, it, :])
                if not split_store:
                    nc.sync.dma_start(out_v, osb[:])

            pending = None
            for b in range(B):
                for p in range(NP):
                    i0, i1 = 2 * p, 2 * p + 1
                    pv = pvps.tile([128, 512], F32, tag="pv", name="pv")
                    smp = sumps.tile([128, 512], F32, tag="sm", name="sm")
                    mvq = qtb[:, b, i0:i0 + 2, :, :]
                    # diagonal k-pair first (its mask latency hides under the
                    # interiors); QK(t+1) is emitted before PV(t) so the PE
                    # queue never head-of-line-blocks on an exp in flight
                    def qk_emit(t):
                        j0, j1 = 2 * t, 2 * t + 1
                        scp = scps.tile([128, 2, 512], F32, tag="sc",
                                        name="scp")
                        probs = probsp.tile([128, 2, 512], BF, tag="probs",
                                            name="probs")
                        if t < p:
                            nc.tensor.matmul(scp[:, 0, :], ktb[:, b, j0, :],
                                             mvq)
                            nc.tensor.matmul(scp[:, 1, :], ktb[:, b, j1, :],
                                             mvq)
                            nc.scalar.activation(probs[:], scp[:], AF.Exp)
                        else:
                            nc.tensor.matmul(scp[:, 0, :], ktb[:, b, j0, :],
                                             mvq)
                            nc.tensor.matmul(scp[:, 1, 0:256],
                                             ktb[:, b, j1, :],
                                             qtb[:, b, i1, :, :])
                            nc.scalar.activation(probs[:, 0, :], scp[:, 0, :],
                                                 AF.Exp)
                            nc.scalar.activation(probs[:, 1, 0:256],
                                                 scp[:, 1, 0:256], AF.Exp)
                            nc.gpsimd.tensor_mul(probs[:, :, 0:256],
                                                 probs[:, :, 0:256], tri_s[:])
                        return probs

                    def pv_emit(t, probs):
                        j0, j1 = 2 * t, 2 * t + 1
                        first = t == 0
                        if t < p:
                            for sub, j in ((0, j0), (1, j1)):
                                nc.tensor.matmul(
                                    pv[:], vb[:, b, j, :], probs[:, sub, :],
                                    start=first and sub == 0, stop=False)
                                nc.tensor.matmul(
                                    smp[:], ones_s[:], probs[:, sub, :],
                                    start=first and sub == 0, stop=False)
                        else:
                            nc.tensor.matmul(pv[:, 0:256], vb[:, b, j0, :],
                                             probs[:, 0, 0:256],
                                             start=first, stop=True)
                            nc.tensor.matmul(pv[:, 256:512], vb[:, b, j0, :],
                                             probs[:, 0, 256:512],
                                             start=first, stop=False)
                            nc.tensor.matmul(pv[:, 256:512], vb[:, b, j1, :],
                                             probs[:, 1, 0:256],
                                             start=False, stop=True)
                            nc.tensor.matmul(smp[:, 0:256], ones_s[:],
                                             probs[:, 0, 0:256],
                                             start=first, stop=True)
                            nc.tensor.matmul(smp[:, 256:512], ones_s[:],
                                             probs[:, 0, 256:512],
                                             start=first, stop=False)
                            nc.tensor.matmul(smp[:, 256:512], ones_s[:],
                                             probs[:, 1, 0:256],
                                             start=False, stop=True)

                    t_seq = list(range(p + 1))
                    pending_pv = (t_seq[0], qk_emit(t_seq[0]))
                    for tn in t_seq[1:]:
                        nxt = (tn, qk_emit(tn))
                        pv_emit(*pending_pv)
                        pending_pv = nxt
                    pv_emit(*pending_pv)
                    if pending is not None:
                        wo_proj(*pending)
                    # softmax divide + sigmoid gate; hi-lo gated stash
                    rsb = awork.tile([128, 512], F32, tag="rsb")
                    nc.vector.reciprocal(rsb[:], smp[:])
                    tmp = awork.tile([128, 512], BF, tag="tmp")
                    nc.vector.scalar_tensor_tensor(tmp[:], pv[:], 1.0,
                                                   rsb[:], ALU.mult, ALU.mult)
                    gfull = awork.tile([128, 2, 2, 128], BF, tag="gf")
                    nc.vector.tensor_mul(gfull[:], tmp[:],
                                         gtb[:, b, i0:i0 + 2, :, :])
                    ghi = probsp.tile([128, 2, 2, 128], E4, tag="ghi")
                    nc.vector.tensor_scalar_mul(ghi[:], gfull[:], SG)
                    glo = probsp.tile([128, 2, 2, 128], E4, tag="glo")
                    nc.vector.scalar_tensor_tensor(glo[:], gfull[:], SG,
                                                   ghi[:], ALU.mult,
                                                   ALU.subtract)
                    pending = (b, i0, ghi, glo)
            wo_proj(*pending, split_store=True)
    nc.compile()
    return nc


def prep_inputs(hidden_states, cos, sin, Wq, Wk, Wv, Wo, q_norm_w, k_norm_w,
                S_=S):
    N = B * S_
    hsT = np.ascontiguousarray(
        hidden_states.reshape(N, HID).T).astype(np.float32) * SX
    hsthi = hsT.astype(E4NP)
    hstlo = (hsT - hsthi.astype(np.float32)).astype(E4NP)
    HC = HID // 128
    hsthi = hsthi.reshape(HC, 128, N)
    hstlo = hstlo.reshape(HC, 128, N)

    cos0 = np.asarray(cos[0], np.float32)
    sin0 = np.asarray(sin[0], np.float32)
    qw = np.asarray(q_norm_w, np.float32)
    kw = np.asarray(k_norm_w, np.float32)
    sign = np.where(np.arange(D) < 64, -1.0, 1.0).astype(np.float32)
    shift = (np.arange(D) + 64) % D

    cosq = np.ascontiguousarray(cos0.T * qw[:, None] * SCALE).astype(BF16)
    sinq = np.ascontiguousarray(
        sin0.T * (sign * qw[shift])[:, None] * SCALE).astype(BF16)
    cosk = np.ascontiguousarray(cos0.T * kw[:, None]).astype(BF16)
    sink = np.ascontiguousarray(
        sin0.T * (sign * kw[shift])[:, None]).astype(BF16)
    cosq2 = np.ascontiguousarray(np.stack([cosq, cosq], axis=1))
    sinq2 = np.ascontiguousarray(np.stack([sinq, sinq], axis=1))

    # diag mask: probs[:, sub, 0:256] has k-token on partitions and
    # (head, tok) on columns; keep k <= q i.e. p <= col % 128
    toks = np.arange(256) % 128
    tri2 = np.ascontiguousarray(np.stack(
        [(np.arange(128)[:, None] <= toks[None, :]).astype(BF16)] * 2, axis=1))
    ident = np.eye(128, dtype=BF16)

    in_maps = []
    for d in range(NCORES):
        h0, h1 = G * d, G * d + 1
        cols = [Wq[:, h0 * 2 * D: h0 * 2 * D + D],
                Wq[:, h1 * 2 * D: h1 * 2 * D + D],
                Wk[:, d * D:(d + 1) * D],
                Wv[:, d * D:(d + 1) * D],
                Wq[:, h0 * 2 * D + D: (h0 + 1) * 2 * D],
                Wq[:, h1 * 2 * D + D: (h1 + 1) * 2 * D]]
        wcols = np.concatenate(cols, axis=1).astype(np.float32) * SW
        whi = wcols.astype(E4NP)
        wlo = (wcols - whi.astype(np.float32)).astype(E4NP)

        wo_rows = np.ascontiguousarray(
            Wo[d * G * D:(d + 1) * G * D, :]).astype(np.float32) * SW
        wohi = wo_rows.astype(E4NP)
        wolo = (wo_rows - wohi.astype(np.float32)).astype(E4NP)

        in_maps.append({
            "hsthi": hsthi, "hstlo": hstlo,
            "whi": np.ascontiguousarray(whi).reshape(HC, 128, 768),
            "wlo": np.ascontiguousarray(wlo).reshape(HC, 128, 768),
            "wohi": wohi.reshape(G, 128, HID),
            "wolo": wolo.reshape(G, 128, HID),
            "cosq2": cosq2, "sinq2": sinq2, "cosk": cosk, "sink": sink,
            "tri2": tri2, "ident": ident,
        })
    return in_maps


_NC_CACHE = {}
_RUNNER_CACHE = {}


def _get_nc(S_=S):
    if S_ not in _NC_CACHE:
        _NC_CACHE[S_] = build_nc(S_)
    return _NC_CACHE[S_]


def _get_runner(S_=S):
    if S_ in _RUNNER_CACHE:
        return _RUNNER_CACHE[S_]
    import jax
    from jax.experimental.shard_map import shard_map
    from jax.sharding import Mesh, PartitionSpec
    from concourse import bass2jax, mybir as _mybir
    bass2jax.install_neuronx_cc_hook()

    nc = _get_nc(S_)
    assert nc.dbg_addr is None
    pid_name = (nc.partition_id_tensor.name
                if nc.partition_id_tensor is not None else None)

    in_names, out_names, out_avals = [], [], []
    for alloc in nc.m.functions[0].allocations:
        if not isinstance(alloc, _mybir.MemoryLocationSet):
            continue
        name = alloc.memorylocations[0].name
        if alloc.kind == "ExternalInput":
            if name != pid_name:
                in_names.append(name)
        elif alloc.kind == "ExternalOutput":
            out_names.append(name)
            out_avals.append(jax.core.ShapedArray(
                tuple(alloc.tensor_shape), _mybir.dt.np(alloc.dtype)))
    n_params = len(in_names)
    all_names = in_names + out_names
    if pid_name is not None:
        all_names = all_names + [pid_name]

    def _body(*args):
        operands = list(args)
        if pid_name is not None:
            operands.append(bass2jax.partition_id_tensor())
        outs = bass2jax._bass_exec_p.bind(
            *operands,
            out_avals=tuple(out_avals),
            in_names=tuple(all_names),
            out_names=tuple(out_names),
            lowering_input_output_aliases=(),
            sim_require_finite=True,
            sim_require_nnan=True,
            nc=nc,
        )
        return tuple(outs)

    devices = jax.devices()[:NCORES]
    mesh = Mesh(np.asarray(devices), ("core",))
    nin = n_params + len(out_names)
    sharded = jax.jit(
        shard_map(_body, mesh=mesh,
                  in_specs=(PartitionSpec("core"),) * nin,
                  out_specs=(PartitionSpec("core"),) * len(out_names),
                  check_rep=False),
        keep_unused=True,
    )
    zeros = [np.zeros((NCORES * a.shape[0], *a.shape[1:]), a.dtype)
             for a in out_avals]
    zeros_dev = [jax.device_put(z) for z in zeros]

    def run(in_maps):
        concat_in = [
            np.concatenate([np.asarray(m[nm]) for m in in_maps], axis=0)
            for nm in in_names
        ]
        outs = sharded(*concat_in, *zeros_dev)
        return {nm: np.asarray(outs[i]) for i, nm in enumerate(out_names)}

    def run_prepared(dev_args):
        return sharded(*dev_args, *zeros_dev)

    def prepare(in_maps):
        return [
            jax.device_put(np.concatenate(
                [np.asarray(m[nm]) for m in in_maps], axis=0))
            for nm in in_names
        ]

    r = {"run": run, "prepare": prepare, "run_prepared": run_prepared,
         "out_names": out_names, "out_avals": out_avals}
    _RUNNER_CACHE[S_] = r
    return r


def kernel(hidden_states, cos, sin, Wq, Wk, Wv, Wo, q_norm_w, k_norm_w):
    in_maps = prep_inputs(hidden_states, cos, sin, Wq, Wk, Wv, Wo,
                          q_norm_w, k_norm_w)
    runner = _get_runner()
    outs = runner["run"](in_maps)
    full = outs["out"].reshape(NCORES, B * S, HID)
    acc = full.astype(np.float32).sum(axis=0)
    return acc.reshape(B, S, HID)


# revision 4
# speedup vs baseline: 1.0378x; 1.0034x over previous
"""Qwen-style GQA full attention (B=2, S=2048, HID=2048, H=16, KVH=8, D=128)
on 8 trn2 NeuronCores — v2: hi-lo fp8 DoubleRow matmuls.

Sharding: tensor-parallel across head groups (core d owns kv-head d and its
two query heads). Each core emits a partial [B*S, HID] via its Wo row block;
the host sums the 8 partials in f32.

Numerics: projections and Wo run as error-compensated fp8 ("hi-lo"): each
operand x is pre-scaled into e4m3's normal range and split x ~= hi + lo
(both e4m3); y = hi*Wh + lo*Wh + hi*Wl (3 DoubleRow matmuls, K=256 each,
the lo*lo term ~0.03% is dropped). This measures *better* than bf16
(0.11% vs 0.23% on the projection GEMM) at 1.33x bf16 matmul throughput
(DoubleRow streams 2 fp8 rows/cycle). Attention (QK/PV/denominator-sum)
stays bf16: with random weights the softmax is diffuse, so per-element
quantization noise in q/k/v/probs reaches the output at full strength
(fp8 there measurably busts the 2e-2 budget).

Performance notes (sim cost model):
  - PE sequencer dispatch is ~84ns per matmul (ldweights+matmult); small
    matmuls are sequencer-bound, so every DR matmul moves the full 1024
    fp8 elements/row (512 output cols) and V is projected feature-major
    (512-wide moving) then PE-transposed, instead of token-major 128-wide.
  - GPSIMD (Pool) is Q7 software: ~0.42 efficiency + 95ns launch, and it
    cannot touch PSUM; only SBUF-side work with slack goes there.
  - Hot-path DMAs ride the SP and ACT hardware queues; bulk constants go
    on the Pool queue after the first chunk's loads (the cost model
    serializes all transfers through one DMA device, so early ordering
    decides when the first matmuls can start).
  - RMSNorm uses gpsimd partition_all_reduce (exact, and cheap in a way
    the Q7 C-axis tensor_reduce is not), producing an SBUF-resident
    broadcast sum directly: the whole rstd chain (square, all-reduce, Ln,
    Exp, apply) runs off-PE, so no broadcast matmuls and no PE-queue
    dependency on it at all.
  - sigmoid(g) = 1/(1+exp(-g)): ACT exp + DVE add/reciprocal, keeping the
    ACT table set at {Ln, Exp, Copy} (one table load, no thrash).
"""

import numpy as np
import ml_dtypes

import concourse.bass as bass
import concourse.bass_isa as bass_isa
import concourse.tile as tile
from concourse import bacc, mybir
from contextlib import ExitStack

BF16 = ml_dtypes.bfloat16
E4NP = ml_dtypes.float8_e4m3
F32 = mybir.dt.float32
BF = mybir.dt.bfloat16
E4 = mybir.dt.float8e4
AF = mybir.ActivationFunctionType
DR = mybir.MatmulPerfMode.DoubleRow
ALU = mybir.AluOpType


class _Bacc(bacc.Bacc):
    """Pin the combined Ln+Exp activation table set (see module docstring)."""

    def insert_act_table_loads(self):
        import bass_rust as _bass_rust
        from concourse.hw_specs import get_activation_tables
        has_activation = any(
            isinstance(i, mybir.InstActivation)
            for b in self.main_func.blocks
            for i in b.instructions
        )
        if not has_activation:
            return
        items = [
            (nm, fns if nm == "natural_log_exp_and_others" else set())
            for nm, fns in get_activation_tables(self.m.arch).items()
        ]
        _bass_rust.insert_act_table_loads(self, items)


B, S, HID, H, KVH, D = 2, 2048, 2048, 16, 8, 128
G = H // KVH
EPS = 1e-6
SCALE = D ** -0.5
CH = 512
NCORES = 8

SX = 16.0        # hidden-state fp8 pre-scale
SW = 128.0       # weight fp8 pre-scale
SG = 32.0        # gated-output fp8 pre-scale
IXW = 1.0 / (SX * SW)
IGW = 1.0 / (SG * SW)

# packed projection-weight column layout: q0 q1 k v g0 g1 (128 each)
COLS = {"q0": 0, "q1": 128, "k": 256, "v": 384, "g0": 512, "g1": 640}


def build_nc(S_=S):
    HC = HID // 128
    N = B * S_
    SK = S_ // 128
    NP = S_ // 256
    CPB = S_ // CH
    NT = CH // 128
    NKP = HC // 2

    nc = _Bacc(None)

    hsthi_d = nc.dram_tensor("hsthi", [HC, 128, N], E4, kind="ExternalInput")
    hstlo_d = nc.dram_tensor("hstlo", [HC, 128, N], E4, kind="ExternalInput")
    whi_d = nc.dram_tensor("whi", [HC, 128, 768], E4, kind="ExternalInput")
    wlo_d = nc.dram_tensor("wlo", [HC, 128, 768], E4, kind="ExternalInput")
    wohi_d = nc.dram_tensor("wohi", [G, 128, HID], E4, kind="ExternalInput")
    wolo_d = nc.dram_tensor("wolo", [G, 128, HID], E4, kind="ExternalInput")
    cq2_d = nc.dram_tensor("cosq2", [128, 2, S_], BF, kind="ExternalInput")
    sq2_d = nc.dram_tensor("sinq2", [128, 2, S_], BF, kind="ExternalInput")
    ck_d = nc.dram_tensor("cosk", [128, S_], BF, kind="ExternalInput")
    sk_d = nc.dram_tensor("sink", [128, S_], BF, kind="ExternalInput")
    tri_d = nc.dram_tensor("tri2", [128, 2, 256], BF, kind="ExternalInput")
    id_d = nc.dram_tensor("ident", [128, 128], BF, kind="ExternalInput")
    out_d = nc.dram_tensor("out", [N, HID], BF, kind="ExternalOutput")

    with tile.TileContext(nc) as tc, ExitStack() as ctx:
        cpool = ctx.enter_context(tc.tile_pool(name="consts", bufs=1))

        whi_s = cpool.tile([128, HC, 768], E4)
        wlo_s = cpool.tile([128, HC, 768], E4)
        wohi_s = cpool.tile([128, G, HID], E4)
        wolo_s = cpool.tile([128, G, HID], E4)
        cq2_s = cpool.tile([128, 2, S_], BF)
        sq2_s = cpool.tile([128, 2, S_], BF)
        ck_s = cpool.tile([128, S_], BF)
        sk_s = cpool.tile([128, S_], BF)
        tri_s = cpool.tile([128, 2, 256], BF)
        id_s = cpool.tile([128, 128], BF)
        ones_s = cpool.tile([128, 128], BF)
        o1_s = cpool.tile([128, 1], BF)
        ob_s = cpool.tile([1, 128], BF)
        epsb = cpool.tile([128, 1], F32)
        nc.vector.memset(ones_s[:], 1.0)
        nc.vector.memset(o1_s[:], 1.0)
        nc.vector.memset(ob_s[:], 1.0)
        nc.vector.memset(epsb[:], EPS)

        # weight loads interleaved on ACT (the first matmuls need whi+wlo);
        # tables/Wo weights ride the Pool queue (slack early, needed later)
        whi_v = whi_d[:].rearrange("c p f -> p c f")
        wlo_v = wlo_d[:].rearrange("c p f -> p c f")
        nc.scalar.dma_start(whi_s[:, 0:4, :], whi_v[:, 0:4, :])
        nc.scalar.dma_start(wlo_s[:, 0:4, :], wlo_v[:, 0:4, :])

        # persistent activations
        qtb = cpool.tile([128, B, SK, G, 128], BF)   # rope'd+normed q (feat-major)
        ktb = cpool.tile([128, B, SK, 128], BF)      # rope'd+normed k (feat-major)
        vtb = cpool.tile([128, N], BF)               # v feature-major staging
        vb = cpool.tile([128, B, SK, 128], BF)       # v token-major
        gtb = cpool.tile([128, B, SK, G, 128], F32)  # sigmoid(gate)

        # ---------------- phase 1: projections (hi-lo fp8 DR) ----------------
        # Blocks are emitted sequentially (all 24 accumulating matmuls of one
        # output block, then its consumers) so each PSUM bank is freed ~13us
        # before the next chunk needs it, and consumer work spreads evenly.
        with (
            tc.tile_pool(name="hst", bufs=2) as hstp,
            tc.tile_pool(name="qgps", bufs=3, space="PSUM") as qgps,
            tc.tile_pool(name="auxps", bufs=2, space="PSUM") as auxps,
            tc.tile_pool(name="pwork", bufs=2) as pwork,
            tc.tile_pool(name="pw1", bufs=1) as pw1,
        ):
            for b in range(B):
                for cc in range(CPB):
                    t0 = b * S_ + cc * CH
                    p0 = cc * CH
                    ti0 = cc * NT
                    first_chunk = b == 0 and cc == 0
                    hh = hstp.tile([128, HC, CH], E4, tag="hh")
                    hl = hstp.tile([128, HC, CH], E4, tag="hl")
                    if first_chunk:
                        for c4 in range(4, HC, 4):
                            nc.scalar.dma_start(whi_s[:, c4:c4 + 4, :],
                                                whi_v[:, c4:c4 + 4, :])
                            nc.scalar.dma_start(wlo_s[:, c4:c4 + 4, :],
                                                wlo_v[:, c4:c4 + 4, :])
                        # tables after chunk-0 weights: needed only by the
                        # first rope/mask consumers, keep them off the
                        # critical early DMA window
                        for dst, srct in ((cq2_s, cq2_d), (sq2_s, sq2_d),
                                          (ck_s, ck_d), (sk_s, sk_d),
                                          (tri_s, tri_d), (id_s, id_d)):
                            nc.gpsimd.dma_start(dst[:], srct[:])
                        nc.gpsimd.dma_start(
                            wohi_s[:], wohi_d[:].rearrange("c p f -> p c f"))
                        nc.gpsimd.dma_start(
                            wolo_s[:], wolo_d[:].rearrange("c p f -> p c f"))
                    step = 2 if first_chunk else 8
                    for c4 in range(0, HC, step):
                        nc.sync.dma_start(
                            hh[:, c4:c4 + step, :],
                            hsthi_d[c4:c4 + step, :, t0:t0 + CH].rearrange(
                                "c p f -> p c f"))
                        nc.sync.dma_start(
                            hl[:, c4:c4 + step, :],
                            hstlo_d[c4:c4 + step, :, t0:t0 + CH].rearrange(
                                "c p f -> p c f"))

                    xus = {}
                    ssts = {}
                    e1 = pw1.tile([128, 2, CH], BF, tag="e1")
                    s12s = {}

                    def run_blocks(specs):
                        # specs: [(nm, n_prods)]; for the DMA-bound first
                        # chunk the blocks advance kp-inner together so
                        # matmuls track ht chunk arrival instead of waiting
                        # for the whole tensor
                        pss, prodss = [], []
                        for nm, n_prods in specs:
                            pss.append(qgps.tile([128, CH], F32, tag="pp",
                                                 name=f"ps_{nm}"))
                            prodss.append(
                                [(whi_s, hh), (whi_s, hl),
                                 (wlo_s, hh)][:n_prods])
                        for kp in range(NKP):
                            c = 2 * kp
                            for (nm, _), ps, prods in zip(specs, pss, prodss):
                                col0 = COLS[nm]
                                for pi, (wsrc, hsrc) in enumerate(prods):
                                    nc.tensor.matmul(
                                        ps[:],
                                        wsrc[:, c:c + 2, col0:col0 + 128],
                                        hsrc[:, c:c + 2, :],
                                        start=kp == 0 and pi == 0,
                                        stop=(kp == NKP - 1
                                              and pi == len(prods) - 1),
                                        perf_mode=DR)
                        return pss

                    def run_block(nm, n_prods):
                        return run_blocks([(nm, n_prods)])[0]

                    def norm_stats(nm, i, ps):
                        xu = pwork.tile([128, CH], BF, tag=f"xu_{nm}",
                                        name="xu")
                        nc.scalar.activation(xu[:], ps[:], AF.Copy, scale=IXW)
                        xus[nm] = xu
                        xsq = pw1.tile([128, CH], BF, tag=f"xsq_{nm}",
                                       name="xsq")
                        nc.vector.tensor_mul(xsq[:], xu[:], xu[:])
                        ssB = pw1.tile([128, CH], F32, tag=f"ssB_{nm}",
                                       name="ssB")
                        nc.gpsimd.partition_all_reduce(
                            ssB[:], xsq[:], 128, bass_isa.ReduceOp.add)
                        lnB = pw1.tile([128, CH], BF, tag=f"lnB_{nm}",
                                       name="lnB")
                        nc.scalar.activation(lnB[:], ssB[:], AF.Ln,
                                             bias=epsb[:], scale=1.0 / D)
                        rstdB = pw1.tile([128, CH], BF, tag=f"rstdB_{nm}",
                                         name="rstdB")
                        nc.scalar.activation(rstdB[:], lnB[:], AF.Exp,
                                             scale=-0.5)
                        ssts[nm] = rstdB

                    def rope_sum(nm, i, ctab, stab, cidx):
                        # 2-input SBUF ops need equal base partitions, so the
                        # half-rotation is a pair of 1-input copies first.
                        xu = xus[nm]
                        t1 = pw1.tile([128, CH], BF, tag=f"t1_{nm}", name="t1")
                        nc.vector.tensor_mul(t1[:], xu[:],
                                             ctab[:, cidx, p0:p0 + CH]
                                             if cidx is not None
                                             else ctab[:, p0:p0 + CH])
                        xr = pw1.tile([128, CH], BF, tag=f"xr_{nm}", name="xr")
                        nc.vector.tensor_copy(xr[0:64, :], xu[64:128, :])
                        nc.vector.tensor_copy(xr[64:128, :], xu[0:64, :])
                        t2 = pw1.tile([128, CH], BF, tag=f"t2_{nm}", name="t2")
                        sv = (stab[:, cidx, p0:p0 + CH] if cidx is not None
                              else stab[:, p0:p0 + CH])
                        nc.gpsimd.tensor_mul(t2[:], xr[:], sv)
                        s12 = pw1.tile([128, CH], BF, tag=f"s12_{nm}",
                                       name="s12")
                        nc.vector.tensor_add(s12[:], t1[:], t2[:])
                        s12s[nm] = s12

                    def emit_bc(nm, s12, dest):
                        nc.vector.scalar_tensor_tensor(dest, s12[:], 1.0,
                                                       ssts[nm][:],
                                                       ALU.mult, ALU.mult)

                    tabs = {"q0": (cq2_s, sq2_s, 0), "q1": (cq2_s, sq2_s, 1),
                            "k": (ck_s, sk_s, None)}
                    names3 = ("q0", "q1", "k")

                    dests = {"q0": qtb[:, b, ti0:ti0 + NT, 0, :],
                             "q1": qtb[:, b, ti0:ti0 + NT, 1, :],
                             "k": ktb[:, b, ti0:ti0 + NT, :]}

                    def qcons(nm):
                        i = names3.index(nm)
                        norm_stats(nm, i, pss[nm])
                        rope_sum(nm, i, *tabs[nm])
                        emit_bc(nm, s12s[nm], dests[nm])

                    def vcons():
                        nc.scalar.activation(vtb[:, t0:t0 + CH], pss["v"][:],
                                             AF.Copy, scale=IXW)

                    def gcons(h):
                        nc.scalar.activation(e1[:, h, :],
                                             pss["g0" if h == 0 else "g1"][:],
                                             AF.Exp, scale=-IXW)

                    pss = {}
                    if first_chunk:
                        # DMA-bound: advance pairs of blocks kp-inner so the
                        # PE tracks ht chunk arrival
                        for grp in ([("q0", 3), ("q1", 3)],
                                    [("k", 3), ("v", 3)],
                                    [("g0", 2), ("g1", 2)]):
                            res = run_blocks(grp)
                            pss.update({nm: ps for (nm, _), ps
                                        in zip(grp, res)})
                            if grp[0][0] == "q0":
                                qcons("q0")
                                qcons("q1")
                            elif grp[0][0] == "k":
                                qcons("k")
                                vcons()
                            else:
                                gcons(0)
                                gcons(1)
                    else:
                        for nm in names3:
                            pss[nm] = run_block(nm, 3)
                            qcons(nm)
                        pss["v"] = run_block("v", 3)
                        vcons()
                        pss["g0"] = run_block("g0", 2)
                        gcons(0)
                        pss["g1"] = run_block("g1", 2)
                        gcons(1)

                    # sigmoid: 1/(1+e1) -> gtb (f32)
                    a1 = pw1.tile([128, 2, CH], F32, tag="a1")
                    nc.vector.tensor_scalar_add(a1[:], e1[:], 1.0)
                    for h in range(G):
                        nc.vector.reciprocal(
                            gtb[:, b, ti0:ti0 + NT, h, :], a1[:, h, :])

                # V -> token-major for this batch (PE transposes)
                for j4 in range(0, SK, 4):
                    vt_ps = auxps.tile([128, 512], BF, tag="aux", name="vt")
                    for jj in range(4):
                        j = j4 + jj
                        nc.tensor.transpose(
                            vt_ps[:, jj * 128:(jj + 1) * 128],
                            vtb[:, b * S_ + j * 128:b * S_ + (j + 1) * 128],
                            id_s[:])
                    nc.vector.tensor_copy(vb[:, b, j4:j4 + 4, :], vt_ps[:])

        # ---------------- phase 2: attention + gating + Wo ----------------
        with (
            tc.tile_pool(name="scps", bufs=2, space="PSUM") as scps,
            tc.tile_pool(name="pvps", bufs=1, space="PSUM") as pvps,
            tc.tile_pool(name="sumps", bufs=1, space="PSUM") as sumps,
            tc.tile_pool(name="wops", bufs=2, space="PSUM") as wops,
            tc.tile_pool(name="probsp", bufs=4) as probsp,
            tc.tile_pool(name="awork", bufs=3) as awork,
        ):
            def wo_proj(b, i0, ghi, glo, split_store=False):
                osb = awork.tile([128, 2, HID], BF, tag="osb")
                trow = b * S_ + i0 * 128
                out_v = out_d[trow:trow + 256, :].rearrange(
                    "(a p) f -> p a f", p=128)
                for it in range(2):
                    for oc in range(4):
                        wop = wops.tile([128, 512], F32, tag="wo")
                        for pi, (gs, ws) in enumerate(
                                ((ghi, wohi_s), (glo, wohi_s), (ghi, wolo_s))):
                            nc.tensor.matmul(
                                wop[:], gs[:, it, :, :],
                                ws[:, :, oc * 512:(oc + 1) * 512],
                                start=pi == 0, stop=pi == 2, perf_mode=DR)
                        dst = osb[:, it, oc * 512:(oc + 1) * 512]
                        if oc % 2 == 0:
                            nc.vector.tensor_scalar_mul(dst, wop[:], IGW)
                        else:
                            nc.scalar.activation(dst, wop[:], AF.Copy,
                                                 scale=IGW)
                    if split_store:
                        q = nc.sync if it == 0 else nc.gpsimd
                        q.dma_start(out_v[:, it, :], osb[:, it, :])
                if not split_store:
                    nc.sync.dma_start(out_v, osb[:])

            pending = None
            for b in range(B):
                for p in range(NP):
                    i0, i1 = 2 * p, 2 * p + 1
                    pv = pvps.tile([128, 512], F32, tag="pv", name="pv")
                    smp = sumps.tile([128, 512], F32, tag="sm", name="sm")
                    mvq = qtb[:, b, i0:i0 + 2, :, :]
                    # diagonal k-pair first (its mask latency hides under the
                    # interiors); QK(t+1) is emitted before PV(t) so the PE
                    # queue never head-of-line-blocks on an exp in flight
                    def qk_emit(t):
                        j0, j1 = 2 * t, 2 * t + 1
                        scp = scps.tile([128, 2, 512], F32, tag="sc",
                                        name="scp")
                        probs = probsp.tile([128, 2, 512], BF, tag="probs",
                                            name="probs")
                        if t < p:
                            nc.tensor.matmul(scp[:, 0, :], ktb[:, b, j0, :],
                                             mvq)
                            nc.tensor.matmul(scp[:, 1, :], ktb[:, b, j1, :],
                                             mvq)
                            nc.scalar.activation(probs[:], scp[:], AF.Exp)
                        else:
                            nc.tensor.matmul(scp[:, 0, :], ktb[:, b, j0, :],
                                             mvq)
                            nc.tensor.matmul(scp[:, 1, 0:256],
                                             ktb[:, b, j1, :],
                                             qtb[:, b, i1, :, :])
                            nc.scalar.activation(probs[:, 0, :], scp[:, 0, :],
                                                 AF.Exp)
                            nc.scalar.activation(probs[:, 1, 0:256],
                                                 scp[:, 1, 0:256], AF.Exp)
                            nc.gpsimd.tensor_mul(probs[:, :, 0:256],
                                                 probs[:, :, 0:256], tri_s[:])
                        return probs

                    def pv_emit(t, probs):
                        j0, j1 = 2 * t, 2 * t + 1
                        first = t == 0
                        if t < p:
                            for sub, j in ((0, j0), (1, j1)):
                                nc.tensor.matmul(
                                    pv[:], vb[:, b, j, :], probs[:, sub, :],
                                    start=first and sub == 0, stop=False)
                                nc.tensor.matmul(
                                    smp[:], ones_s[:], probs[:, sub, :],
                                    start=first and sub == 0, stop=False)
                        else:
                            nc.tensor.matmul(pv[:, 0:256], vb[:, b, j0, :],
                                             probs[:, 0, 0:256],
                                             start=first, stop=True)
                            nc.tensor.matmul(pv[:, 256:512], vb[:, b, j0, :],
                                             probs[:, 0, 256:512],
                                             start=first, stop=False)
                            nc.tensor.matmul(pv[:, 256:512], vb[:, b, j1, :],
                                             probs[:, 1, 0:256],
                                             start=False, stop=True)
                            nc.tensor.matmul(smp[:, 0:256], ones_s[:],
                                             probs[:, 0, 0:256],
                                             start=first, stop=True)
                            nc.tensor.matmul(smp[:, 256:512], ones_s[:],
                                             probs[:, 0, 256:512],
                                             start=first, stop=False)
                            nc.tensor.matmul(smp[:, 256:512], ones_s[:],
                                             probs[:, 1, 0:256],
                                             start=False, stop=True)

                    t_seq = list(range(p + 1))
                    pending_pv = (t_seq[0], qk_emit(t_seq[0]))
                    for tn in t_seq[1:]:
                        nxt = (tn, qk_emit(tn))
                        pv_emit(*pending_pv)
                        pending_pv = nxt
                    pv_emit(*pending_pv)
                    if pending is not None:
                        wo_proj(*pending)
                    # softmax divide + sigmoid gate; hi-lo gated stash
                    rsb = awork.tile([128, 512], F32, tag="rsb")
                    nc.vector.reciprocal(rsb[:], smp[:])
                    tmp = awork.tile([128, 512], BF, tag="tmp")
                    nc.vector.scalar_tensor_tensor(tmp[:], pv[:], 1.0,
                                                   rsb[:], ALU.mult, ALU.mult)
                    gfull = awork.tile([128, 2, 2, 128], BF, tag="gf")
                    nc.vector.tensor_mul(gfull[:], tmp[:],
                                         gtb[:, b, i0:i0 + 2, :, :])
                    ghi = probsp.tile([128, 2, 2, 128], E4, tag="ghi")
                    nc.vector.tensor_scalar_mul(ghi[:], gfull[:], SG)
                    glo = probsp.tile([128, 2, 2, 128], E4, tag="glo")
                    nc.vector.scalar_tensor_tensor(glo[:], gfull[:], SG,
                                                   ghi[:], ALU.mult,
                                                   ALU.subtract)
                    pending = (b, i0, ghi, glo)
            wo_proj(*pending, split_store=True)
    nc.compile()
    return nc


def prep_inputs(hidden_states, cos, sin, Wq, Wk, Wv, Wo, q_norm_w, k_norm_w,
                S_=S):
    N = B * S_
    hsT = np.ascontiguousarray(
        hidden_states.reshape(N, HID).T).astype(np.float32) * SX
    hsthi = hsT.astype(E4NP)
    hstlo = (hsT - hsthi.astype(np.float32)).astype(E4NP)
    HC = HID // 128
    hsthi = hsthi.reshape(HC, 128, N)
    hstlo = hstlo.reshape(HC, 128, N)

    cos0 = np.asarray(cos[0], np.float32)
    sin0 = np.asarray(sin[0], np.float32)
    qw = np.asarray(q_norm_w, np.float32)
    kw = np.asarray(k_norm_w, np.float32)
    sign = np.where(np.arange(D) < 64, -1.0, 1.0).astype(np.float32)
    shift = (np.arange(D) + 64) % D

    cosq = np.ascontiguousarray(cos0.T * qw[:, None] * SCALE).astype(BF16)
    sinq = np.ascontiguousarray(
        sin0.T * (sign * qw[shift])[:, None] * SCALE).astype(BF16)
    cosk = np.ascontiguousarray(cos0.T * kw[:, None]).astype(BF16)
    sink = np.ascontiguousarray(
        sin0.T * (sign * kw[shift])[:, None]).astype(BF16)
    cosq2 = np.ascontiguousarray(np.stack([cosq, cosq], axis=1))
    sinq2 = np.ascontiguousarray(np.stack([sinq, sinq], axis=1))

    # diag mask: probs[:, sub, 0:256] has k-token on partitions and
    # (head, tok) on columns; keep k <= q i.e. p <= col % 128
    toks = np.arange(256) % 128
    tri2 = np.ascontiguousarray(np.stack(
        [(np.arange(128)[:, None] <= toks[None, :]).astype(BF16)] * 2, axis=1))
    ident = np.eye(128, dtype=BF16)

    in_maps = []
    for d in range(NCORES):
        h0, h1 = G * d, G * d + 1
        cols = [Wq[:, h0 * 2 * D: h0 * 2 * D + D],
                Wq[:, h1 * 2 * D: h1 * 2 * D + D],
                Wk[:, d * D:(d + 1) * D],
                Wv[:, d * D:(d + 1) * D],
                Wq[:, h0 * 2 * D + D: (h0 + 1) * 2 * D],
                Wq[:, h1 * 2 * D + D: (h1 + 1) * 2 * D]]
        wcols = np.concatenate(cols, axis=1).astype(np.float32) * SW
        whi = wcols.astype(E4NP)
        wlo = (wcols - whi.astype(np.float32)).astype(E4NP)

        wo_rows = np.ascontiguousarray(
            Wo[d * G * D:(d + 1) * G * D, :]).astype(np.float32) * SW
        wohi = wo_rows.astype(E4NP)
        wolo = (wo_rows - wohi.astype(np.float32)).astype(E4NP)

        in_maps.append({
            "hsthi": hsthi, "hstlo": hstlo,
            "whi": np.ascontiguousarray(whi).reshape(HC, 128, 768),
            "wlo": np.ascontiguousarray(wlo).reshape(HC, 128, 768),
            "wohi": wohi.reshape(G, 128, HID),
            "wolo": wolo.reshape(G, 128, HID),
            "cosq2": cosq2, "sinq2": sinq2, "cosk": cosk, "sink": sink,
            "tri2": tri2, "ident": ident,
        })
    return in_maps


_NC_CACHE = {}
_RUNNER_CACHE = {}


def _get_nc(S_=S):
    if S_ not in _NC_CACHE:
        _NC_CACHE[S_] = build_nc(S_)
    return _NC_CACHE[S_]


def _get_runner(S_=S):
    if S_ in _RUNNER_CACHE:
        return _RUNNER_CACHE[S_]
    import jax
    from jax.experimental.shard_map import shard_map
    from jax.sharding import Mesh, PartitionSpec
    from concourse import bass2jax, mybir as _mybir
    bass2jax.install_neuronx_cc_hook()

    nc = _get_nc(S_)
    assert nc.dbg_addr is None
    pid_name = (nc.partition_id_tensor.name
                if nc.partition_id_tensor is not None else None)

    in_names, out_names, out_avals = [], [], []
    for alloc in nc.m.functions[0].allocations:
        if not isinstance(alloc, _mybir.MemoryLocationSet):
            continue
        name = alloc.memorylocations[0].name
        if alloc.kind == "ExternalInput":
            if name != pid_name:
                in_names.append(name)
        elif alloc.kind == "ExternalOutput":
            out_names.append(name)
            out_avals.append(jax.core.ShapedArray(
                tuple(alloc.tensor_shape), _mybir.dt.np(alloc.dtype)))
    n_params = len(in_names)
    all_names = in_names + out_names
    if pid_name is not None:
        all_names = all_names + [pid_name]

    def _body(*args):
        operands = list(args)
        if pid_name is not None:
            operands.append(bass2jax.partition_id_tensor())
        outs = bass2jax._bass_exec_p.bind(
            *operands,
            out_avals=tuple(out_avals),
            in_names=tuple(all_names),
            out_names=tuple(out_names),
            lowering_input_output_aliases=(),
            sim_require_finite=True,
            sim_require_nnan=True,
            nc=nc,
        )
        return tuple(outs)

    devices = jax.devices()[:NCORES]
    mesh = Mesh(np.asarray(devices), ("core",))
    nin = n_params + len(out_names)
    sharded = jax.jit(
        shard_map(_body, mesh=mesh,
                  in_specs=(PartitionSpec("core"),) * nin,
                  out_specs=(PartitionSpec("core"),) * len(out_names),
                  check_rep=False),
        keep_unused=True,
    )
    zeros = [np.zeros((NCORES * a.shape[0], *a.shape[1:]), a.dtype)
             for a in out_avals]
    zeros_dev = [jax.device_put(z) for z in zeros]

    def run(in_maps):
        concat_in = [
            np.concatenate([np.asarray(m[nm]) for m in in_maps], axis=0)
            for nm in in_names
        ]
        outs = sharded(*concat_in, *zeros_dev)
        return {nm: np.asarray(outs[i]) for i, nm in enumerate(out_names)}

    def run_prepared(dev_args):
        return sharded(*dev_args, *zeros_dev)

    def prepare(in_maps):
        return [
            jax.device_put(np.concatenate(
                [np.asarray(m[nm]) for m in in_maps], axis=0))
            for nm in in_names
        ]

    r = {"run": run, "prepare": prepare, "run_prepared": run_prepared,
         "out_names": out_names, "out_avals": out_avals}
    _RUNNER_CACHE[S_] = r
    return r


def kernel(hidden_states, cos, sin, Wq, Wk, Wv, Wo, q_norm_w, k_norm_w):
    in_maps = prep_inputs(hidden_states, cos, sin, Wq, Wk, Wv, Wo,
                          q_norm_w, k_norm_w)
    runner = _get_runner()
    outs = runner["run"](in_maps)
    full = outs["out"].reshape(NCORES, B * S, HID)
    acc = full.astype(np.float32).sum(axis=0)
    return acc.reshape(B, S, HID)


# revision 5
# speedup vs baseline: 1.0465x; 1.0084x over previous
"""Qwen-style GQA full attention (B=2, S=2048, HID=2048, H=16, KVH=8, D=128)
on 8 trn2 NeuronCores — v2: hi-lo fp8 DoubleRow matmuls.

Sharding: tensor-parallel across head groups (core d owns kv-head d and its
two query heads). Each core emits a partial [B*S, HID] via its Wo row block;
the host sums the 8 partials in f32.

Numerics: projections and Wo run as error-compensated fp8 ("hi-lo"): each
operand x is pre-scaled into e4m3's normal range and split x ~= hi + lo
(both e4m3); y = hi*Wh + lo*Wh + hi*Wl (3 DoubleRow matmuls, K=256 each,
the lo*lo term ~0.03% is dropped). This measures *better* than bf16
(0.11% vs 0.23% on the projection GEMM) at 1.33x bf16 matmul throughput
(DoubleRow streams 2 fp8 rows/cycle). Attention (QK/PV/denominator-sum)
stays bf16: with random weights the softmax is diffuse, so per-element
quantization noise in q/k/v/probs reaches the output at full strength
(fp8 there measurably busts the 2e-2 budget).

Performance notes (sim cost model):
  - PE sequencer dispatch is ~84ns per matmul (ldweights+matmult); small
    matmuls are sequencer-bound, so every DR matmul moves the full 1024
    fp8 elements/row (512 output cols) and V is projected feature-major
    (512-wide moving) then PE-transposed, instead of token-major 128-wide.
  - GPSIMD (Pool) is Q7 software: ~0.42 efficiency + 95ns launch, and it
    cannot touch PSUM; only SBUF-side work with slack goes there.
  - Hot-path DMAs ride the SP and ACT hardware queues; bulk constants go
    on the Pool queue after the first chunk's loads (the cost model
    serializes all transfers through one DMA device, so early ordering
    decides when the first matmuls can start).
  - RMSNorm uses gpsimd partition_all_reduce (exact, and cheap in a way
    the Q7 C-axis tensor_reduce is not), producing an SBUF-resident
    broadcast sum directly: the whole rstd chain (square, all-reduce, Ln,
    Exp, apply) runs off-PE, so no broadcast matmuls and no PE-queue
    dependency on it at all.
  - sigmoid(g) = 1/(1+exp(-g)): ACT exp + DVE add/reciprocal, keeping the
    ACT table set at {Ln, Exp, Copy} (one table load, no thrash).
"""

import numpy as np
import ml_dtypes

import concourse.bass as bass
import concourse.bass_isa as bass_isa
import concourse.tile as tile
from concourse import bacc, mybir
from contextlib import ExitStack

BF16 = ml_dtypes.bfloat16
E4NP = ml_dtypes.float8_e4m3
F32 = mybir.dt.float32
BF = mybir.dt.bfloat16
E4 = mybir.dt.float8e4
AF = mybir.ActivationFunctionType
DR = mybir.MatmulPerfMode.DoubleRow
ALU = mybir.AluOpType


class _Bacc(bacc.Bacc):
    """Pin the combined Ln+Exp activation table set (see module docstring)."""

    def insert_act_table_loads(self):
        import bass_rust as _bass_rust
        from concourse.hw_specs import get_activation_tables
        has_activation = any(
            isinstance(i, mybir.InstActivation)
            for b in self.main_func.blocks
            for i in b.instructions
        )
        if not has_activation:
            return
        items = [
            (nm, fns if nm == "natural_log_exp_and_others" else set())
            for nm, fns in get_activation_tables(self.m.arch).items()
        ]
        _bass_rust.insert_act_table_loads(self, items)


B, S, HID, H, KVH, D = 2, 2048, 2048, 16, 8, 128
G = H // KVH
EPS = 1e-6
SCALE = D ** -0.5
CH = 512
NCORES = 8

SX = 16.0        # hidden-state fp8 pre-scale
SW = 128.0       # weight fp8 pre-scale
SG = 32.0        # gated-output fp8 pre-scale
IXW = 1.0 / (SX * SW)
IGW = 1.0 / (SG * SW)

# packed projection-weight column layout: q0 q1 k v g0 g1 (128 each)
COLS = {"q0": 0, "q1": 128, "k": 256, "v": 384, "g0": 512, "g1": 640}


def build_nc(S_=S):
    HC = HID // 128
    N = B * S_
    SK = S_ // 128
    NP = S_ // 256
    CPB = S_ // CH
    NT = CH // 128
    NKP = HC // 2

    nc = _Bacc(None)

    hsthi_d = nc.dram_tensor("hsthi", [HC, 128, N], E4, kind="ExternalInput")
    hstlo_d = nc.dram_tensor("hstlo", [HC, 128, N], E4, kind="ExternalInput")
    whi_d = nc.dram_tensor("whi", [HC, 128, 768], E4, kind="ExternalInput")
    wlo_d = nc.dram_tensor("wlo", [HC, 128, 768], E4, kind="ExternalInput")
    wohi_d = nc.dram_tensor("wohi", [G, 128, HID], E4, kind="ExternalInput")
    wolo_d = nc.dram_tensor("wolo", [G, 128, HID], E4, kind="ExternalInput")
    cq_d = nc.dram_tensor("cosq", [128, S_], BF, kind="ExternalInput")
    sq_d = nc.dram_tensor("sinq", [128, S_], BF, kind="ExternalInput")
    ck_d = nc.dram_tensor("cosk", [128, S_], BF, kind="ExternalInput")
    sk_d = nc.dram_tensor("sink", [128, S_], BF, kind="ExternalInput")
    tri_d = nc.dram_tensor("tri2", [128, 2, 256], BF, kind="ExternalInput")
    id_d = nc.dram_tensor("ident", [128, 128], BF, kind="ExternalInput")
    out_d = nc.dram_tensor("out", [N, HID], BF, kind="ExternalOutput")

    with tile.TileContext(nc) as tc, ExitStack() as ctx:
        cpool = ctx.enter_context(tc.tile_pool(name="consts", bufs=1))

        whi_s = cpool.tile([128, HC, 768], E4)
        wlo_s = cpool.tile([128, HC, 768], E4)
        wohi_s = cpool.tile([128, G, HID], E4)
        wolo_s = cpool.tile([128, G, HID], E4)
        cq_s = cpool.tile([128, S_], BF)
        sq_s = cpool.tile([128, S_], BF)
        ck_s = cpool.tile([128, S_], BF)
        sk_s = cpool.tile([128, S_], BF)
        tri_s = cpool.tile([128, 2, 256], BF)
        id_s = cpool.tile([128, 128], BF)
        ones_s = cpool.tile([128, 128], BF)
        o1_s = cpool.tile([128, 1], BF)
        ob_s = cpool.tile([1, 128], BF)
        epsb = cpool.tile([128, 1], F32)
        nc.vector.memset(ones_s[:], 1.0)
        nc.vector.memset(o1_s[:], 1.0)
        nc.vector.memset(ob_s[:], 1.0)
        nc.vector.memset(epsb[:], EPS)

        # weight loads interleaved on ACT (the first matmuls need whi+wlo);
        # tables/Wo weights ride the Pool queue (slack early, needed later)
        whi_v = whi_d[:].rearrange("c p f -> p c f")
        wlo_v = wlo_d[:].rearrange("c p f -> p c f")
        nc.scalar.dma_start(whi_s[:, 0:4, :], whi_v[:, 0:4, :])
        nc.scalar.dma_start(wlo_s[:, 0:4, :], wlo_v[:, 0:4, :])

        # persistent activations
        qtb = cpool.tile([128, B, SK, G, 128], BF)   # rope'd+normed q (feat-major)
        ktb = cpool.tile([128, B, SK, 128], BF)      # rope'd+normed k (feat-major)
        vtb = cpool.tile([128, N], BF)               # v feature-major staging
        vb = cpool.tile([128, B, SK, 128], BF)       # v token-major
        gtb = cpool.tile([128, B, SK, G, 128], F32)  # sigmoid(gate)

        # ---------------- phase 1: projections (hi-lo fp8 DR) ----------------
        # Blocks are emitted sequentially (all 24 accumulating matmuls of one
        # output block, then its consumers) so each PSUM bank is freed ~13us
        # before the next chunk needs it, and consumer work spreads evenly.
        with (
            tc.tile_pool(name="hst", bufs=2) as hstp,
            tc.tile_pool(name="qgps", bufs=3, space="PSUM") as qgps,
            tc.tile_pool(name="auxps", bufs=2, space="PSUM") as auxps,
            tc.tile_pool(name="pwork", bufs=2) as pwork,
            tc.tile_pool(name="pw1", bufs=1) as pw1,
        ):
            for b in range(B):
                for cc in range(CPB):
                    t0 = b * S_ + cc * CH
                    p0 = cc * CH
                    ti0 = cc * NT
                    first_chunk = b == 0 and cc == 0
                    hh = hstp.tile([128, HC, CH], E4, tag="hh")
                    hl = hstp.tile([128, HC, CH], E4, tag="hl")
                    if first_chunk:
                        for c4 in range(4, HC, 4):
                            nc.scalar.dma_start(whi_s[:, c4:c4 + 4, :],
                                                whi_v[:, c4:c4 + 4, :])
                            nc.scalar.dma_start(wlo_s[:, c4:c4 + 4, :],
                                                wlo_v[:, c4:c4 + 4, :])
                        # tables after chunk-0 weights: needed only by the
                        # first rope/mask consumers, keep them off the
                        # critical early DMA window
                        for dst, srct in ((cq_s, cq_d), (sq_s, sq_d),
                                          (ck_s, ck_d), (sk_s, sk_d),
                                          (tri_s, tri_d), (id_s, id_d)):
                            nc.gpsimd.dma_start(dst[:], srct[:])
                        nc.gpsimd.dma_start(
                            wohi_s[:], wohi_d[:].rearrange("c p f -> p c f"))
                        nc.gpsimd.dma_start(
                            wolo_s[:], wolo_d[:].rearrange("c p f -> p c f"))
                    step = 2 if first_chunk else 8
                    for c4 in range(0, HC, step):
                        nc.sync.dma_start(
                            hh[:, c4:c4 + step, :],
                            hsthi_d[c4:c4 + step, :, t0:t0 + CH].rearrange(
                                "c p f -> p c f"))
                        nc.sync.dma_start(
                            hl[:, c4:c4 + step, :],
                            hstlo_d[c4:c4 + step, :, t0:t0 + CH].rearrange(
                                "c p f -> p c f"))

                    xus = {}
                    ssts = {}
                    e1 = pw1.tile([128, 2, CH], BF, tag="e1")
                    s12s = {}

                    def run_blocks(specs):
                        # specs: [(nm, n_prods)]; for the DMA-bound first
                        # chunk the blocks advance kp-inner together so
                        # matmuls track ht chunk arrival instead of waiting
                        # for the whole tensor
                        pss, prodss = [], []
                        for nm, n_prods in specs:
                            pss.append(qgps.tile([128, CH], F32, tag="pp",
                                                 name=f"ps_{nm}"))
                            prodss.append(
                                [(whi_s, hh), (whi_s, hl),
                                 (wlo_s, hh)][:n_prods])
                        for kp in range(NKP):
                            c = 2 * kp
                            for (nm, _), ps, prods in zip(specs, pss, prodss):
                                col0 = COLS[nm]
                                for pi, (wsrc, hsrc) in enumerate(prods):
                                    nc.tensor.matmul(
                                        ps[:],
                                        wsrc[:, c:c + 2, col0:col0 + 128],
                                        hsrc[:, c:c + 2, :],
                                        start=kp == 0 and pi == 0,
                                        stop=(kp == NKP - 1
                                              and pi == len(prods) - 1),
                                        perf_mode=DR)
                        return pss

                    def run_block(nm, n_prods):
                        return run_blocks([(nm, n_prods)])[0]

                    def norm_stats(nm, i, ps):
                        xu = pwork.tile([128, CH], BF, tag=f"xu_{nm}",
                                        name="xu")
                        nc.scalar.activation(xu[:], ps[:], AF.Copy, scale=IXW)
                        xus[nm] = xu
                        xsq = pw1.tile([128, CH], BF, tag=f"xsq_{nm}",
                                       name="xsq")
                        nc.vector.tensor_mul(xsq[:], xu[:], xu[:])
                        ssB = pw1.tile([128, CH], F32, tag=f"ssB_{nm}",
                                       name="ssB")
                        nc.gpsimd.partition_all_reduce(
                            ssB[:], xsq[:], 128, bass_isa.ReduceOp.add)
                        lnB = pw1.tile([128, CH], BF, tag=f"lnB_{nm}",
                                       name="lnB")
                        nc.scalar.activation(lnB[:], ssB[:], AF.Ln,
                                             bias=epsb[:], scale=1.0 / D)
                        rstdB = pw1.tile([128, CH], BF, tag=f"rstdB_{nm}",
                                         name="rstdB")
                        nc.scalar.activation(rstdB[:], lnB[:], AF.Exp,
                                             scale=-0.5)
                        ssts[nm] = rstdB

                    def rope_sum(nm, i, ctab, stab, cidx):
                        # 2-input SBUF ops need equal base partitions, so the
                        # half-rotation is a pair of 1-input copies first.
                        xu = xus[nm]
                        t1 = pw1.tile([128, CH], BF, tag=f"t1_{nm}", name="t1")
                        nc.vector.tensor_mul(t1[:], xu[:],
                                             ctab[:, cidx, p0:p0 + CH]
                                             if cidx is not None
                                             else ctab[:, p0:p0 + CH])
                        xr = pw1.tile([128, CH], BF, tag=f"xr_{nm}", name="xr")
                        nc.vector.tensor_copy(xr[0:64, :], xu[64:128, :])
                        nc.vector.tensor_copy(xr[64:128, :], xu[0:64, :])
                        t2 = pw1.tile([128, CH], BF, tag=f"t2_{nm}", name="t2")
                        sv = (stab[:, cidx, p0:p0 + CH] if cidx is not None
                              else stab[:, p0:p0 + CH])
                        nc.gpsimd.tensor_mul(t2[:], xr[:], sv)
                        s12 = pw1.tile([128, CH], BF, tag=f"s12_{nm}",
                                       name="s12")
                        nc.vector.tensor_add(s12[:], t1[:], t2[:])
                        s12s[nm] = s12

                    def emit_bc(nm, s12, dest):
                        nc.vector.scalar_tensor_tensor(dest, s12[:], 1.0,
                                                       ssts[nm][:],
                                                       ALU.mult, ALU.mult)

                    tabs = {"q0": (cq_s, sq_s, None),
                            "q1": (cq_s, sq_s, None),
                            "k": (ck_s, sk_s, None)}
                    names3 = ("q0", "q1", "k")

                    dests = {"q0": qtb[:, b, ti0:ti0 + NT, 0, :],
                             "q1": qtb[:, b, ti0:ti0 + NT, 1, :],
                             "k": ktb[:, b, ti0:ti0 + NT, :]}

                    def qcons(nm):
                        i = names3.index(nm)
                        norm_stats(nm, i, pss[nm])
                        rope_sum(nm, i, *tabs[nm])
                        emit_bc(nm, s12s[nm], dests[nm])

                    def vcons():
                        nc.scalar.activation(vtb[:, t0:t0 + CH], pss["v"][:],
                                             AF.Copy, scale=IXW)

                    def gcons(h):
                        nc.scalar.activation(e1[:, h, :],
                                             pss["g0" if h == 0 else "g1"][:],
                                             AF.Exp, scale=-IXW)

                    pss = {}
                    if first_chunk:
                        # DMA-bound: advance pairs of blocks kp-inner so the
                        # PE tracks ht chunk arrival
                        for grp in ([("q0", 3), ("q1", 3)],
                                    [("k", 3), ("v", 3)],
                                    [("g0", 2), ("g1", 2)]):
                            res = run_blocks(grp)
                            pss.update({nm: ps for (nm, _), ps
                                        in zip(grp, res)})
                            if grp[0][0] == "q0":
                                qcons("q0")
                                qcons("q1")
                            elif grp[0][0] == "k":
                                qcons("k")
                                vcons()
                            else:
                                gcons(0)
                                gcons(1)
                    else:
                        for nm in names3:
                            pss[nm] = run_block(nm, 3)
                            qcons(nm)
                        pss["v"] = run_block("v", 3)
                        vcons()
                        pss["g0"] = run_block("g0", 2)
                        gcons(0)
                        pss["g1"] = run_block("g1", 2)
                        gcons(1)

                    # sigmoid: 1/(1+e1) -> gtb (f32)
                    a1 = pw1.tile([128, 2, CH], F32, tag="a1")
                    nc.vector.tensor_scalar_add(a1[:], e1[:], 1.0)
                    for h in range(G):
                        nc.vector.reciprocal(
                            gtb[:, b, ti0:ti0 + NT, h, :], a1[:, h, :])

                # V -> token-major for this batch (PE transposes)
                for j4 in range(0, SK, 4):
                    vt_ps = auxps.tile([128, 512], BF, tag="aux", name="vt")
                    for jj in range(4):
                        j = j4 + jj
                        nc.tensor.transpose(
                            vt_ps[:, jj * 128:(jj + 1) * 128],
                            vtb[:, b * S_ + j * 128:b * S_ + (j + 1) * 128],
                            id_s[:])
                    nc.vector.tensor_copy(vb[:, b, j4:j4 + 4, :], vt_ps[:])

        # ---------------- phase 2: attention + gating + Wo ----------------
        with (
            tc.tile_pool(name="scps", bufs=2, space="PSUM") as scps,
            tc.tile_pool(name="pvps", bufs=1, space="PSUM") as pvps,
            tc.tile_pool(name="sumps", bufs=1, space="PSUM") as sumps,
            tc.tile_pool(name="wops", bufs=2, space="PSUM") as wops,
            tc.tile_pool(name="probsp", bufs=4) as probsp,
            tc.tile_pool(name="awork", bufs=3) as awork,
        ):
            def wo_proj(b, i0, ghi, glo, split_store=False, its=(0, 1)):
                osb = awork.tile([128, 2, HID], BF, tag="osb")
                trow = b * S_ + i0 * 128
                out_v = out_d[trow:trow + 256, :].rearrange(
                    "(a p) f -> p a f", p=128)
                for it in its:
                    for oc in range(4):
                        wop = wops.tile([128, 512], F32, tag="wo")
                        for pi, (gs, ws) in enumerate(
                                ((ghi, wohi_s), (glo, wohi_s), (ghi, wolo_s))):
                            nc.tensor.matmul(
                                wop[:], gs[:, it, :, :],
                                ws[:, :, oc * 512:(oc + 1) * 512],
                                start=pi == 0, stop=pi == 2, perf_mode=DR)
                        dst = osb[:, it, oc * 512:(oc + 1) * 512]
                        if oc % 2 == 0:
                            nc.vector.tensor_scalar_mul(dst, wop[:], IGW)
                        else:
                            nc.scalar.activation(dst, wop[:], AF.Copy,
                                                 scale=IGW)
                    if split_store:
                        q = nc.sync if it == 0 else nc.gpsimd
                        q.dma_start(out_v[:, it, :], osb[:, it, :])
                if not split_store:
                    nc.sync.dma_start(out_v, osb[:])

            pending = None
            for b in range(B):
                for p in range(NP):
                    i0, i1 = 2 * p, 2 * p + 1
                    pv = pvps.tile([128, 512], F32, tag="pv", name="pv")
                    smp = sumps.tile([128, 512], F32, tag="sm", name="sm")
                    mvq = qtb[:, b, i0:i0 + 2, :, :]
                    # diagonal k-pair first (its mask latency hides under the
                    # interiors); QK(t+1) is emitted before PV(t) so the PE
                    # queue never head-of-line-blocks on an exp in flight
                    def qk_emit(t):
                        j0, j1 = 2 * t, 2 * t + 1
                        scp = scps.tile([128, 2, 512], F32, tag="sc",
                                        name="scp")
                        probs = probsp.tile([128, 2, 512], BF, tag="probs",
                                            name="probs")
                        if t < p:
                            nc.tensor.matmul(scp[:, 0, :], ktb[:, b, j0, :],
                                             mvq)
                            nc.tensor.matmul(scp[:, 1, :], ktb[:, b, j1, :],
                                             mvq)
                            nc.scalar.activation(probs[:], scp[:], AF.Exp)
                        else:
                            nc.tensor.matmul(scp[:, 0, :], ktb[:, b, j0, :],
                                             mvq)
                            nc.tensor.matmul(scp[:, 1, 0:256],
                                             ktb[:, b, j1, :],
                                             qtb[:, b, i1, :, :])
                            nc.scalar.activation(probs[:, 0, :], scp[:, 0, :],
                                                 AF.Exp)
                            nc.scalar.activation(probs[:, 1, 0:256],
                                                 scp[:, 1, 0:256], AF.Exp)
                            nc.gpsimd.tensor_mul(probs[:, :, 0:256],
                                                 probs[:, :, 0:256], tri_s[:])
                        return probs

                    def pv_emit(t, probs):
                        j0, j1 = 2 * t, 2 * t + 1
                        first = t == 0
                        if t < p:
                            for sub, j in ((0, j0), (1, j1)):
                                nc.tensor.matmul(
                                    pv[:], vb[:, b, j, :], probs[:, sub, :],
                                    start=first and sub == 0, stop=False)
                                nc.tensor.matmul(
                                    smp[:], ones_s[:], probs[:, sub, :],
                                    start=first and sub == 0, stop=False)
                        else:
                            nc.tensor.matmul(pv[:, 0:256], vb[:, b, j0, :],
                                             probs[:, 0, 0:256],
                                             start=first, stop=True)
                            nc.tensor.matmul(pv[:, 256:512], vb[:, b, j0, :],
                                             probs[:, 0, 256:512],
                                             start=first, stop=False)
                            nc.tensor.matmul(pv[:, 256:512], vb[:, b, j1, :],
                                             probs[:, 1, 0:256],
                                             start=False, stop=True)
                            nc.tensor.matmul(smp[:, 0:256], ones_s[:],
                                             probs[:, 0, 0:256],
                                             start=first, stop=True)
                            nc.tensor.matmul(smp[:, 256:512], ones_s[:],
                                             probs[:, 0, 256:512],
                                             start=first, stop=False)
                            nc.tensor.matmul(smp[:, 256:512], ones_s[:],
                                             probs[:, 1, 0:256],
                                             start=False, stop=True)

                    t_seq = list(range(p + 1))
                    pending_pv = (t_seq[0], qk_emit(t_seq[0]))
                    for tn in t_seq[1:]:
                        nxt = (tn, qk_emit(tn))
                        pv_emit(*pending_pv)
                        pending_pv = nxt
                    pv_emit(*pending_pv)
                    if pending is not None:
                        wo_proj(*pending)
                    # softmax divide + sigmoid gate; hi-lo gated stash
                    rsb = awork.tile([128, 512], F32, tag="rsb")
                    nc.vector.reciprocal(rsb[:], smp[:])
                    tmp = awork.tile([128, 512], BF, tag="tmp")
                    nc.vector.scalar_tensor_tensor(tmp[:], pv[:], 1.0,
                                                   rsb[:], ALU.mult, ALU.mult)
                    gfull = awork.tile([128, 2, 2, 128], BF, tag="gf")
                    nc.vector.tensor_mul(gfull[:], tmp[:],
                                         gtb[:, b, i0:i0 + 2, :, :])
                    ghi = probsp.tile([128, 2, 2, 128], E4, tag="ghi")
                    nc.vector.tensor_scalar_mul(ghi[:], gfull[:], SG)
                    glo = probsp.tile([128, 2, 2, 128], E4, tag="glo")
                    nc.vector.scalar_tensor_tensor(glo[:], gfull[:], SG,
                                                   ghi[:], ALU.mult,
                                                   ALU.subtract)
                    pending = (b, i0, ghi, glo)
            wo_proj(*pending, split_store=True)
    nc.compile()
    return nc


def prep_inputs(hidden_states, cos, sin, Wq, Wk, Wv, Wo, q_norm_w, k_norm_w,
                S_=S):
    N = B * S_
    hsT = np.ascontiguousarray(
        hidden_states.reshape(N, HID).T).astype(np.float32) * SX
    hsthi = hsT.astype(E4NP)
    hstlo = (hsT - hsthi.astype(np.float32)).astype(E4NP)
    HC = HID // 128
    hsthi = hsthi.reshape(HC, 128, N)
    hstlo = hstlo.reshape(HC, 128, N)

    cos0 = np.asarray(cos[0], np.float32)
    sin0 = np.asarray(sin[0], np.float32)
    qw = np.asarray(q_norm_w, np.float32)
    kw = np.asarray(k_norm_w, np.float32)
    sign = np.where(np.arange(D) < 64, -1.0, 1.0).astype(np.float32)
    shift = (np.arange(D) + 64) % D

    cosq = np.ascontiguousarray(cos0.T * qw[:, None] * SCALE).astype(BF16)
    sinq = np.ascontiguousarray(
        sin0.T * (sign * qw[shift])[:, None] * SCALE).astype(BF16)
    cosk = np.ascontiguousarray(cos0.T * kw[:, None]).astype(BF16)
    sink = np.ascontiguousarray(
        sin0.T * (sign * kw[shift])[:, None]).astype(BF16)

    # diag mask: probs[:, sub, 0:256] has k-token on partitions and
    # (head, tok) on columns; keep k <= q i.e. p <= col % 128
    toks = np.arange(256) % 128
    tri2 = np.ascontiguousarray(np.stack(
        [(np.arange(128)[:, None] <= toks[None, :]).astype(BF16)] * 2, axis=1))
    ident = np.eye(128, dtype=BF16)

    in_maps = []
    for d in range(NCORES):
        h0, h1 = G * d, G * d + 1
        cols = [Wq[:, h0 * 2 * D: h0 * 2 * D + D],
                Wq[:, h1 * 2 * D: h1 * 2 * D + D],
                Wk[:, d * D:(d + 1) * D],
                Wv[:, d * D:(d + 1) * D],
                Wq[:, h0 * 2 * D + D: (h0 + 1) * 2 * D],
                Wq[:, h1 * 2 * D + D: (h1 + 1) * 2 * D]]
        wcols = np.concatenate(cols, axis=1).astype(np.float32) * SW
        whi = wcols.astype(E4NP)
        wlo = (wcols - whi.astype(np.float32)).astype(E4NP)

        wo_rows = np.ascontiguousarray(
            Wo[d * G * D:(d + 1) * G * D, :]).astype(np.float32) * SW
        wohi = wo_rows.astype(E4NP)
        wolo = (wo_rows - wohi.astype(np.float32)).astype(E4NP)

        in_maps.append({
            "hsthi": hsthi, "hstlo": hstlo,
            "whi": np.ascontiguousarray(whi).reshape(HC, 128, 768),
            "wlo": np.ascontiguousarray(wlo).reshape(HC, 128, 768),
            "wohi": wohi.reshape(G, 128, HID),
            "wolo": wolo.reshape(G, 128, HID),
            "cosq": cosq, "sinq": sinq, "cosk": cosk, "sink": sink,
            "tri2": tri2, "ident": ident,
        })
    return in_maps


_NC_CACHE = {}
_RUNNER_CACHE = {}


def _get_nc(S_=S):
    if S_ not in _NC_CACHE:
        _NC_CACHE[S_] = build_nc(S_)
    return _NC_CACHE[S_]


def _get_runner(S_=S):
    if S_ in _RUNNER_CACHE:
        return _RUNNER_CACHE[S_]
    import jax
    from jax.experimental.shard_map import shard_map
    from jax.sharding import Mesh, PartitionSpec
    from concourse import bass2jax, mybir as _mybir
    bass2jax.install_neuronx_cc_hook()

    nc = _get_nc(S_)
    assert nc.dbg_addr is None
    pid_name = (nc.partition_id_tensor.name
                if nc.partition_id_tensor is not None else None)

    in_names, out_names, out_avals = [], [], []
    for alloc in nc.m.functions[0].allocations:
        if not isinstance(alloc, _mybir.MemoryLocationSet):
            continue
        name = alloc.memorylocations[0].name
        if alloc.kind == "ExternalInput":
            if name != pid_name:
                in_names.append(name)
        elif alloc.kind == "ExternalOutput":
            out_names.append(name)
            out_avals.append(jax.core.ShapedArray(
                tuple(alloc.tensor_shape), _mybir.dt.np(alloc.dtype)))
    n_params = len(in_names)
    all_names = in_names + out_names
    if pid_name is not None:
        all_names = all_names + [pid_name]

    def _body(*args):
        operands = list(args)
        if pid_name is not None:
            operands.append(bass2jax.partition_id_tensor())
        outs = bass2jax._bass_exec_p.bind(
            *operands,
            out_avals=tuple(out_avals),
            in_names=tuple(all_names),
            out_names=tuple(out_names),
            lowering_input_output_aliases=(),
            sim_require_finite=True,
            sim_require_nnan=True,
            nc=nc,
        )
        return tuple(outs)

    devices = jax.devices()[:NCORES]
    mesh = Mesh(np.asarray(devices), ("core",))
    nin = n_params + len(out_names)
    sharded = jax.jit(
        shard_map(_body, mesh=mesh,
                  in_specs=(PartitionSpec("core"),) * nin,
                  out_specs=(PartitionSpec("core"),) * len(out_names),
                  check_rep=False),
        keep_unused=True,
    )
    zeros = [np.zeros((NCORES * a.shape[0], *a.shape[1:]), a.dtype)
             for a in out_avals]
    zeros_dev = [jax.device_put(z) for z in zeros]

    def run(in_maps):
        concat_in = [
            np.concatenate([np.asarray(m[nm]) for m in in_maps], axis=0)
            for nm in in_names
        ]
        outs = sharded(*concat_in, *zeros_dev)
        return {nm: np.asarray(outs[i]) for i, nm in enumerate(out_names)}

    def run_prepared(dev_args):
        return sharded(*dev_args, *zeros_dev)

    def prepare(in_maps):
        return [
            jax.device_put(np.concatenate(
                [np.asarray(m[nm]) for m in in_maps], axis=0))
            for nm in in_names
        ]

    r = {"run": run, "prepare": prepare, "run_prepared": run_prepared,
         "out_names": out_names, "out_avals": out_avals}
    _RUNNER_CACHE[S_] = r
    return r


def kernel(hidden_states, cos, sin, Wq, Wk, Wv, Wo, q_norm_w, k_norm_w):
    in_maps = prep_inputs(hidden_states, cos, sin, Wq, Wk, Wv, Wo,
                          q_norm_w, k_norm_w)
    runner = _get_runner()
    outs = runner["run"](in_maps)
    full = outs["out"].reshape(NCORES, B * S, HID)
    acc = full.astype(np.float32).sum(axis=0)
    return acc.reshape(B, S, HID)
